# revision 9
# baseline (speedup 1.0000x reference)
"""Bass/Trainium2 kernel for a 12-layer GPT-style transformer (nn_BERT).

Strategy: data-parallel over batch (B=8 -> 1 sequence per NeuronCore).
Each core runs all 12 layers on x^T [D=768, S=512] in "transposed"
activation layout (feature dim on partitions), f32r matmul datapath.

kernel(**inputs) takes the FULL unsharded inputs (as produced by
reference.setup_inputs()) and returns the full [8, 512, 768] output.
"""
import contextlib
import os
import sys
import types

sys.path.insert(0, "/opt/trn_rl_repo")
os.environ.setdefault("JAX_PLATFORMS", "axon")

import numpy as np

import concourse.bass as bass
import concourse.mybir as mybir
import concourse.tile as tile
from concourse import bacc
from concourse import bass_utils

F32 = mybir.dt.float32
F32R = mybir.dt.float32r
AF = mybir.ActivationFunctionType
OP = mybir.AluOpType

B, S, D, H, L, V = 8, 512, 768, 12, 12, 40478
DH = D // H            # 64
DF = 4 * D             # 3072
KC = D // 128          # 6 chunks of the model dim
KF = DF // 128         # 24 chunks of the ffn dim
SC = S // 128          # 4 chunks of the sequence
EPS = 1e-5

N_CORES = 8


def _install_ntff_hook():
    """Register the axon NTFF profiling hook that this image's antenv lacks."""
    if "antenv.axon_hooks" in sys.modules:
        return
    try:
        mod = types.ModuleType("antenv.axon_hooks")
        _h = [None]
        mod.set_axon_ntff_profile_hook = lambda h: _h.__setitem__(0, h)
        mod.get_axon_ntff_profile_hook = lambda: _h[0]
        sys.modules["antenv.axon_hooks"] = mod
        import antenv

        antenv.axon_hooks = mod
        if "/root/.axon_site" not in sys.path:
            sys.path.insert(0, "/root/.axon_site")
        from trn_agent_boot.trn_boot import _ntff_profile_via_ctypes

        mod.set_axon_ntff_profile_hook(
            _ntff_profile_via_ctypes("/opt/axon/libaxon_pjrt.so")
        )
    except Exception:
        pass


def build_program(n_layers=L, phases="ABCLD"):
    nc = bacc.Bacc("TRN2", target_bir_lowering=False, debug=False,
                   num_devices=N_CORES)

    d = {}
    d["x0"] = nc.dram_tensor("x0T", (D, S), F32R, kind="ExternalInput").ap()
    d["wqkv"] = nc.dram_tensor("wqkv", (n_layers, D, 3 * D), F32R, kind="ExternalInput").ap()
    d["bqkv"] = nc.dram_tensor("bqkv", (n_layers, 3 * D), F32, kind="ExternalInput").ap()
    d["wproj"] = nc.dram_tensor("wproj", (n_layers, D, D), F32R, kind="ExternalInput").ap()
    d["bproj"] = nc.dram_tensor("bproj", (n_layers, D), F32, kind="ExternalInput").ap()
    d["g1"] = nc.dram_tensor("g1", (n_layers, D), F32, kind="ExternalInput").ap()
    d["b1"] = nc.dram_tensor("b1", (n_layers, D), F32, kind="ExternalInput").ap()
    d["wfc"] = nc.dram_tensor("wfc", (n_layers, D, DF), F32R, kind="ExternalInput").ap()
    d["bfc"] = nc.dram_tensor("bfc", (n_layers, DF), F32, kind="ExternalInput").ap()
    d["wpr"] = nc.dram_tensor("wpr", (n_layers, DF, D), F32R, kind="ExternalInput").ap()
    d["bpr"] = nc.dram_tensor("bpr", (n_layers, D), F32, kind="ExternalInput").ap()
    d["g2"] = nc.dram_tensor("g2", (n_layers, D), F32, kind="ExternalInput").ap()
    d["b2"] = nc.dram_tensor("b2", (n_layers, D), F32, kind="ExternalInput").ap()
    d["triu"] = nc.dram_tensor("triu", (128, 128), F32R, kind="ExternalInput").ap()
    d["ones_row"] = nc.dram_tensor("ones_row", (1, 128), F32R, kind="ExternalInput").ap()
    d["ones_red"] = nc.dram_tensor("ones_red", (128, 2), F32R, kind="ExternalInput").ap()
    d["out"] = nc.dram_tensor("out", (D, S), F32R, kind="ExternalOutput").ap()

    with tile.TileContext(nc) as tc, \
         nc.allow_low_precision(reason="f32r datapath; rel-err budget 2e-2"):
        _emit(tc, nc, n_layers, d, phases)
    nc.compile()
    return nc


def _emit(tc, nc, n_layers, d, phases="ABCLD"):
    ctx = contextlib.ExitStack()

    # --- long-lived SBUF pools -------------------------------------------
    consts = ctx.enter_context(tc.tile_pool(name="consts", bufs=1))
    # unions: tags share a slot across phases of a layer (qT->nT, kT->res1,
    # v->res2); aT and x get their own.
    uni = ctx.enter_context(tc.tile_pool(name="uni", bufs=1))
    x_pool = ctx.enter_context(tc.tile_pool(name="xp", bufs=2))
    probs_pool = ctx.enter_context(tc.tile_pool(name="probs", bufs=2))
    stats_pool = ctx.enter_context(tc.tile_pool(name="stats", bufs=2))
    wq_pool = ctx.enter_context(tc.tile_pool(name="wq", bufs=2))
    wp_pool = ctx.enter_context(tc.tile_pool(name="wp", bufs=1))
    wf_pool = ctx.enter_context(tc.tile_pool(name="wf", bufs=2))
    wr_pool = ctx.enter_context(tc.tile_pool(name="wr", bufs=4))
    gelu_pool = ctx.enter_context(tc.tile_pool(name="gelu", bufs=3))
    bias_pool = ctx.enter_context(tc.tile_pool(name="bias", bufs=2))

    # constants
    cn = {}
    cn["triu"] = consts.tile([128, 128], F32R, tag="triu", name="triu")       # triu[p, f] = 1 if p <= f
    nc.sync.dma_start(out=cn["triu"], in_=d["triu"])
    cn["ones1"] = consts.tile([1, 128], F32R, tag="ones1", name="ones1")        # full bcast lhsT (K=1)
    nc.sync.dma_start(out=cn["ones1"], in_=d["ones_row"])
    cn["ones64"] = cn["ones1"][:, 0:64]                          # head bcast lhsT (K=1)
    cn["ones_red"] = consts.tile([128, 2], F32R, tag="ones_red", name="ones_red")     # partition-sum lhsT (M=2)
    nc.sync.dma_start(out=cn["ones_red"], in_=d["ones_red"])
    cn["eps"] = consts.tile([1, 1], F32, tag="eps", name="eps")
    nc.vector.memset(cn["eps"], EPS)

    pools = dict(uni=uni, x=x_pool, probs=probs_pool, stats=stats_pool,
                 wq=wq_pool, wp=wp_pool, wf=wf_pool, wr=wr_pool,
                 gelu=gelu_pool, bias=bias_pool)

    # residual stream x^T, [128, KC, S] (chunk-major)
    x_cur = x_pool.tile([128, KC, S], F32R, tag="x")
    nc.sync.dma_start(out=x_cur, in_=d["x0"].rearrange("(k p) s -> p k s", p=128))

    for l in range(n_layers):
        with nc.named_scope(f"layer{l}"):
            x_cur = _layer(tc, nc, l, x_cur, d, cn, pools, phases)

    nc.sync.dma_start(out=d["out"].rearrange("(k p) s -> p k s", p=128), in_=x_cur)
    ctx.close()


def _ld_bias(nc, pool, dram_ap, tag, width):
    t = pool.tile([128, width], F32, tag=tag, name=tag)
    nc.sync.dma_start(out=t, in_=dram_ap.rearrange("(c p) -> p c", p=128))
    return t


def _layer(tc, nc, l, x_cur, d, cn, pools, phases="ABCLD"):
    uni = pools["uni"]; stats_pool = pools["stats"]; bias_pool = pools["bias"]

    # ---- biases / ln params for this layer ------------------------------
    bqkv_t = _ld_bias(nc, bias_pool, d["bqkv"][l], "bqkv", 3 * D // 128)
    bv_b = bias_pool.tile([128, D], F32, tag="bvb")      # V bias, row-bcast
    nc.sync.dma_start(out=bv_b, in_=d["bqkv"][l, 2 * D:3 * D].partition_broadcast(128))
    bproj_t = _ld_bias(nc, bias_pool, d["bproj"][l], "bproj", KC)
    g1_t = _ld_bias(nc, bias_pool, d["g1"][l], "g1", KC)
    b1_t = _ld_bias(nc, bias_pool, d["b1"][l], "b1", KC)
    bfc_t = _ld_bias(nc, bias_pool, d["bfc"][l], "bfc", KF)
    bpr_t = _ld_bias(nc, bias_pool, d["bpr"][l], "bpr", KC)
    g2_t = _ld_bias(nc, bias_pool, d["g2"][l], "g2", KC)
    b2_t = _ld_bias(nc, bias_pool, d["b2"][l], "b2", KC)

    # =====================================================================
    # Phase A: qkv.  q^T,k^T transposed [128, KC, S]; V natural [sk, h, dh].
    # wqkv streamed in 6 column-parts of 384 (parts 0-3: Q,K; 4-5: V).
    # =====================================================================
    qT = uni.tile([128, KC, S], F32R, tag="u_q")         # -> nT later
    kT = uni.tile([128, KC, S], F32R, tag="u_k")         # -> res1 later
    v_nat = uni.tile([128, SC, H, DH], F32R, tag="u_v")  # -> res2 later

    with tc.tile_pool(name="ps_qk", bufs=3, space="PSUM") as ps_qk, \
         tc.tile_pool(name="ps_v", bufs=2, space="PSUM") as ps_v:
        for p in range(4):                 # Q/K parts: columns [384p, 384p+384)
            wpart = pools["wq"].tile([128, KC, 384], F32R, tag="wqkv")
            nc.sync.dma_start(
                out=wpart,
                in_=d["wqkv"][l, :, 384 * p:384 * (p + 1)].rearrange(
                    "(k q) n -> q k n", q=128))
            for j in range(3):
                oc = 3 * p + j             # output chunk of qk^T, 0..11
                pt = ps_qk.tile([128, S], F32)
                for k in range(KC):
                    nc.tensor.matmul(pt, wpart[:, k, 128 * j:128 * (j + 1)],
                                     x_cur[:, k, :], start=(k == 0),
                                     stop=(k == KC - 1))
                dst = qT[:, oc, :] if oc < KC else kT[:, oc - KC, :]
                nc.vector.tensor_scalar(out=dst, in0=pt,
                                        scalar1=bqkv_t[:, oc:oc + 1],
                                        scalar2=None, op0=OP.add)
        for p in range(4, 6):              # V parts: v-features [384(p-4), +384)
            wpart = pools["wq"].tile([128, KC, 384], F32R, tag="wqkv")
            nc.sync.dma_start(
                out=wpart,
                in_=d["wqkv"][l, :, 384 * p:384 * (p + 1)].rearrange(
                    "(k q) n -> q k n", q=128))
            n0 = 384 * (p - 4)
            for sc in range(SC):
                pv = ps_v.tile([128, 384], F32, tag="pv")
                for k in range(KC):
                    nc.tensor.matmul(pv, x_cur[:, k, 128 * sc:128 * (sc + 1)],
                                     wpart[:, k, :], start=(k == 0),
                                     stop=(k == KC - 1))
                # v_nat[:, sc, h, :] = pv + bias_v for heads n0/64 .. n0/64+5
                h0 = n0 // DH
                nc.vector.tensor_tensor(
                    out=v_nat[:, sc, h0:h0 + 6, :],
                    in0=pv.rearrange("q (h e) -> q h e", e=DH),
                    in1=bv_b[:, n0:n0 + 384].rearrange("q (h e) -> q h e", e=DH),
                    op=OP.add)

    if "B" not in phases:
        return qT
    # =====================================================================
    # Phase B: attention, head by head.  scores^T chunks via K^T.T @ Q^T.
    # =====================================================================
    aT = uni.tile([128, KC, S], F32R, tag="u_a")
    with tc.tile_pool(name="ps_sc", bufs=2, space="PSUM") as ps_sc, \
         tc.tile_pool(name="ps_av", bufs=2, space="PSUM") as ps_av, \
         tc.tile_pool(name="ps_dn", bufs=2, space="PSUM") as ps_dn, \
         tc.tile_pool(name="ps_bc", bufs=2, space="PSUM") as ps_bc:
        for h in range(H):
            hc, hh = h // 2, (h % 2) * 64
            probs = pools["probs"].tile([128, SC, S], F32R, tag="probs")
            for c in range(SC):
                n0 = 128 * c                     # sq range [n0, S)
                pt = ps_sc.tile([128, S], F32, tag="score")
                nc.tensor.matmul(pt[:, 0:S - n0],
                                 kT[hh:hh + 64, hc, n0:n0 + 128],
                                 qT[hh:hh + 64, hc, n0:S],
                                 start=True, stop=True)
                nc.scalar.activation(out=probs[:, c, n0:S], in_=pt[:, 0:S - n0],
                                     func=AF.Exp, scale=0.125)
                nc.vector.tensor_tensor(out=probs[:, c, n0:n0 + 128],
                                        in0=probs[:, c, n0:n0 + 128],
                                        in1=cn["triu"], op=OP.mult)
            # denominator: column sums of probs (over sk) via ones_red
            pden = ps_dn.tile([2, S], F32, tag="den")
            for c in range(SC):
                n0 = 128 * c
                nc.tensor.matmul(pden[:, n0:S], cn["ones_red"],
                                 probs[:, c, n0:S],
                                 start=(c == 0), stop=(c == SC - 1),
                                 skip_group_check=True)
            recip = stats_pool.tile([1, S], F32R, tag="recip")
            nc.vector.reciprocal(out=recip, in_=pden[0:1, :])
            # av^T [64, S] accumulated over sk chunks (suffix scheme)
            pav = ps_av.tile([128, S], F32, tag="av")
            for c in range(SC):
                n0 = 128 * c
                nc.tensor.matmul(pav[0:64, n0:S], v_nat[:, c, h, :],
                                 probs[:, c, n0:S],
                                 start=(c == 0), stop=(c == SC - 1),
                                 skip_group_check=True)
            # broadcast 1/den over the head rows, scale, place into aT
            pbc = ps_bc.tile([64, S], F32, tag="bc")
            nc.tensor.matmul(pbc, cn["ones64"], recip, start=True, stop=True)
            bc_s = stats_pool.tile([64, S], F32, tag="bc_s")
            nc.vector.tensor_copy(out=bc_s, in_=pbc)
            if hh == 0:
                nc.vector.tensor_tensor(out=aT[0:64, hc, :], in0=pav[0:64, :],
                                        in1=bc_s, op=OP.mult)
            else:
                av_s = stats_pool.tile([64, S], F32R, tag="av_s")
                nc.vector.tensor_tensor(out=av_s, in0=pav[0:64, :],
                                        in1=bc_s, op=OP.mult)
                nc.sync.dma_start(out=aT[64:128, hc, :], in_=av_s)

    if "C" not in phases:
        return aT
    # =====================================================================
    # Phase C: attn out proj + residual + LN1
    # =====================================================================
    wproj_t = pools["wp"].tile([128, KC, D], F32R, tag="wproj")
    nc.sync.dma_start(out=wproj_t,
                      in_=d["wproj"][l].rearrange("(k p) n -> p k n", p=128))
    res1 = uni.tile([128, KC, S], F32R, tag="u_k")       # reuses kT slot
    with tc.tile_pool(name="ps_pj", bufs=3, space="PSUM") as ps_pj:
        for oc in range(KC):
            pt = ps_pj.tile([128, S], F32)
            for k in range(KC):
                nc.tensor.matmul(pt, wproj_t[:, k, 128 * oc:128 * (oc + 1)],
                                 aT[:, k, :], start=(k == 0), stop=(k == KC - 1))
            nc.vector.tensor_scalar(out=pt, in0=pt, scalar1=bproj_t[:, oc:oc + 1],
                                    scalar2=None, op0=OP.add)
            nc.vector.tensor_tensor(out=res1[:, oc, :], in0=pt,
                                    in1=x_cur[:, oc, :], op=OP.add)

    if "L" not in phases:
        return res1
    nT = uni.tile([128, KC, S], F32R, tag="u_q")         # reuses qT slot
    _layernorm(tc, nc, res1, nT, g1_t, b1_t, cn, stats_pool, "ln1")

    if "D" not in phases:
        return nT
    # =====================================================================
    # Phase D: fused fc -> gelu -> pr (+ residual), k-outer over DF chunks.
    # wfc streamed in 6 column-parts of 512 (4 kf each); wpr in row-slices.
    # =====================================================================
    res2 = uni.tile([128, KC, S], F32R, tag="u_v")       # reuses v_nat slot
    with tc.tile_pool(name="ps_pr", bufs=1, space="PSUM") as ps_pr, \
         tc.tile_pool(name="ps_fc", bufs=2, space="PSUM") as ps_fc:
        pr_acc = [ps_pr.tile([128, S], F32, tag=f"pr{oc}", name=f"pr{oc}") for oc in range(KC)]
        for part in range(6):
            wfc_p = pools["wf"].tile([128, KC, 512], F32R, tag="wfc")
            nc.sync.dma_start(
                out=wfc_p,
                in_=d["wfc"][l, :, 512 * part:512 * (part + 1)].rearrange(
                    "(k q) n -> q k n", q=128))
            for j in range(4):
                kf = 4 * part + j
                wpr_k = pools["wr"].tile([128, D], F32R, tag="wprk")
                nc.sync.dma_start(out=wpr_k,
                                  in_=d["wpr"][l, 128 * kf:128 * (kf + 1), :])
                pf = ps_fc.tile([128, S], F32)
                for k in range(KC):
                    nc.tensor.matmul(pf, wfc_p[:, k, 128 * j:128 * (j + 1)],
                                     nT[:, k, :], start=(k == 0),
                                     stop=(k == KC - 1))
                gk = pools["gelu"].tile([128, S], F32R, tag="gk")
                nc.scalar.activation(out=gk, in_=pf, func=AF.Gelu_apprx_tanh,
                                     bias=bfc_t[:, kf:kf + 1], scale=1.0)
                for oc in range(KC):
                    nc.tensor.matmul(pr_acc[oc],
                                     wpr_k[:, 128 * oc:128 * (oc + 1)],
                                     gk, start=(kf == 0), stop=(kf == KF - 1))
        for oc in range(KC):
            nc.vector.tensor_scalar(out=pr_acc[oc], in0=pr_acc[oc],
                                    scalar1=bpr_t[:, oc:oc + 1],
                                    scalar2=None, op0=OP.add)
            nc.vector.tensor_tensor(out=res2[:, oc, :], in0=pr_acc[oc],
                                    in1=nT[:, oc, :], op=OP.add)

    x_next = pools["x"].tile([128, KC, S], F32R, tag="x")
    _layernorm(tc, nc, res2, x_next, g2_t, b2_t, cn, stats_pool, "ln2")
    return x_next


def _layernorm(tc, nc, src, dst, g_t, b_t, cn, stats_pool, tag):
    """LN over the partition (feature) axis of src [128, KC, S] -> dst."""
    with tc.tile_pool(name=f"ps_{tag}", bufs=1, space="PSUM") as ps:
        psum = ps.tile([2, S], F32, tag="s0")        # row 0: sum(x)
        psq = ps.tile([2, S], F32, tag="s1")         # row 0: sum(x^2)
        for k in range(KC):
            sq = stats_pool.tile([128, S], F32R, tag="lnsq")
            nc.scalar.activation(out=sq, in_=src[:, k, :], func=AF.Square)
            nc.tensor.matmul(psum, cn["ones_red"], src[:, k, :],
                             start=(k == 0), stop=(k == KC - 1))
            nc.tensor.matmul(psq, cn["ones_red"], sq,
                             start=(k == 0), stop=(k == KC - 1))
        mu = stats_pool.tile([1, S], F32R, tag="mu")
        rsd = stats_pool.tile([1, S], F32R, tag="rsd")
        var = stats_pool.tile([1, S], F32, tag="var")
        nc.vector.tensor_scalar(out=mu, in0=psum[0:1, :], scalar1=1.0 / D,
                                scalar2=None, op0=OP.mult)
        nc.vector.tensor_tensor(out=var, in0=mu, in1=mu, op=OP.mult)
        nc.vector.scalar_tensor_tensor(out=var, in0=psq[0:1, :], scalar=1.0 / D,
                                       in1=var, op0=OP.mult, op1=OP.subtract)
        nc.scalar.activation(out=var, in_=var, func=AF.Sqrt, bias=cn["eps"])
        nc.vector.reciprocal(out=rsd, in_=var)
        pmu = ps.tile([128, S], F32, tag="bmu")
        prs = ps.tile([128, S], F32, tag="brs")
        nc.tensor.matmul(pmu, cn["ones1"], mu, start=True, stop=True)
        nc.tensor.matmul(prs, cn["ones1"], rsd, start=True, stop=True)
        for k in range(KC):
            t = stats_pool.tile([128, S], F32, tag="lnt")
            nc.vector.tensor_tensor(out=t, in0=src[:, k, :], in1=pmu,
                                    op=OP.subtract)
            nc.vector.tensor_tensor(out=t, in0=t, in1=prs, op=OP.mult)
            nc.vector.tensor_scalar(out=dst[:, k, :], in0=t,
                                    scalar1=g_t[:, k:k + 1],
                                    scalar2=b_t[:, k:k + 1],
                                    op0=OP.mult, op1=OP.add)


# =========================================================================
# Host side
# =========================================================================
_CACHE = {}


def _get_program():
    if "nc" not in _CACHE:
        _install_ntff_hook()
        _CACHE["nc"] = build_program(L)
    return _CACHE["nc"]


def make_in_maps(inputs, n_layers=L):
    tokens = np.asarray(inputs["tokens"])
    we = np.asarray(inputs["we"], dtype=np.float32)
    pos = we[V:V + S]                                  # [S, D]
    triu = np.triu(np.ones((128, 128), dtype=np.float32))

    def f32(name):
        return np.ascontiguousarray(np.asarray(inputs[name])[:n_layers],
                                    dtype=np.float32)

    shared = {k: f32(k) for k in ["wqkv", "bqkv", "wproj", "bproj", "g1", "b1",
                                  "wfc", "bfc", "wpr", "bpr", "g2", "b2"]}
    shared["triu"] = triu
    shared["ones_row"] = np.ones((1, 128), dtype=np.float32)
    onesred = np.zeros((128, 2), dtype=np.float32); onesred[:, 0] = 1.0
    shared["ones_red"] = onesred
    in_maps = []
    for b in range(N_CORES):
        x0 = we[tokens[b]] + pos                       # [S, D]
        m = dict(shared)
        m["x0T"] = np.ascontiguousarray(x0.T, dtype=np.float32)
        in_maps.append(m)
    return in_maps


def run(inputs, trace=False):
    nc = _get_program()
    in_maps = make_in_maps(inputs)
    res = bass_utils.run_bass_kernel_spmd(nc, in_maps,
                                          core_ids=list(range(N_CORES)),
                                          trace=trace)
    outs = np.stack([res.results[b]["out"].T for b in range(N_CORES)])
    return outs.astype(np.float32), res


def kernel(**inputs):
    out, _ = run(inputs, trace=False)
    return out


# revision 12
# speedup vs baseline: 1.1488x; 1.1488x over previous
"""Bass/Trainium2 kernel for a 12-layer GPT-style transformer (nn_BERT).

Strategy: data-parallel over batch (B=8 -> 1 sequence per NeuronCore).
Each core runs all 12 layers on x^T [D=768, S=512] in "transposed"
activation layout (feature dim on partitions), f32r matmul datapath.

kernel(**inputs) takes the FULL unsharded inputs (as produced by
reference.setup_inputs()) and returns the full [8, 512, 768] output.
"""
import contextlib
import os
import sys
import types

sys.path.insert(0, "/opt/trn_rl_repo")
os.environ.setdefault("JAX_PLATFORMS", "axon")

import numpy as np

import concourse.bass as bass
import concourse.mybir as mybir
import concourse.tile as tile
from concourse import bacc
from concourse import bass_utils

F32 = mybir.dt.float32
F32R = mybir.dt.float32r
AF = mybir.ActivationFunctionType
OP = mybir.AluOpType

B, S, D, H, L, V = 8, 512, 768, 12, 12, 40478
DH = D // H            # 64
DF = 4 * D             # 3072
KC = D // 128          # 6 chunks of the model dim
KF = DF // 128         # 24 chunks of the ffn dim
SC = S // 128          # 4 chunks of the sequence
EPS = 1e-5

N_CORES = 8


def _install_ntff_hook():
    """Register the axon NTFF profiling hook that this image's antenv lacks."""
    if "antenv.axon_hooks" in sys.modules:
        return
    try:
        mod = types.ModuleType("antenv.axon_hooks")
        _h = [None]
        mod.set_axon_ntff_profile_hook = lambda h: _h.__setitem__(0, h)
        mod.get_axon_ntff_profile_hook = lambda: _h[0]
        sys.modules["antenv.axon_hooks"] = mod
        import antenv

        antenv.axon_hooks = mod
        if "/root/.axon_site" not in sys.path:
            sys.path.insert(0, "/root/.axon_site")
        from trn_agent_boot.trn_boot import _ntff_profile_via_ctypes

        mod.set_axon_ntff_profile_hook(
            _ntff_profile_via_ctypes("/opt/axon/libaxon_pjrt.so")
        )
    except Exception:
        pass


def build_program(n_layers=L, phases="ABCLD"):
    nc = bacc.Bacc("TRN2", target_bir_lowering=False, debug=False,
                   num_devices=N_CORES)

    d = {}
    d["x0"] = nc.dram_tensor("x0T", (D, S), F32R, kind="ExternalInput").ap()
    d["wqkv"] = nc.dram_tensor("wqkv", (n_layers, D, 3 * D), F32R, kind="ExternalInput").ap()
    d["bqkv"] = nc.dram_tensor("bqkv", (n_layers, 3 * D), F32, kind="ExternalInput").ap()
    d["wproj"] = nc.dram_tensor("wproj", (n_layers, D, D), F32R, kind="ExternalInput").ap()
    d["bproj"] = nc.dram_tensor("bproj", (n_layers, D), F32, kind="ExternalInput").ap()
    d["g1"] = nc.dram_tensor("g1", (n_layers, D), F32, kind="ExternalInput").ap()
    d["b1"] = nc.dram_tensor("b1", (n_layers, D), F32, kind="ExternalInput").ap()
    d["wfc"] = nc.dram_tensor("wfc", (n_layers, D, DF), F32R, kind="ExternalInput").ap()
    d["bfc"] = nc.dram_tensor("bfc", (n_layers, DF), F32, kind="ExternalInput").ap()
    d["wpr"] = nc.dram_tensor("wpr", (n_layers, DF, D), F32R, kind="ExternalInput").ap()
    d["bpr"] = nc.dram_tensor("bpr", (n_layers, D), F32, kind="ExternalInput").ap()
    d["g2"] = nc.dram_tensor("g2", (n_layers, D), F32, kind="ExternalInput").ap()
    d["b2"] = nc.dram_tensor("b2", (n_layers, D), F32, kind="ExternalInput").ap()
    d["triu"] = nc.dram_tensor("triu", (128, 128), F32R, kind="ExternalInput").ap()
    d["ones_row"] = nc.dram_tensor("ones_row", (1, 128), F32R, kind="ExternalInput").ap()
    d["ones_red"] = nc.dram_tensor("ones_red", (128, 2), F32R, kind="ExternalInput").ap()
    d["sel_den"] = nc.dram_tensor("sel_den", (128, 4, 4), F32R, kind="ExternalInput").ap()
    d["sel_bc"] = nc.dram_tensor("sel_bc", (4, 4, 64), F32R, kind="ExternalInput").ap()
    d["out"] = nc.dram_tensor("out", (D, S), F32R, kind="ExternalOutput").ap()

    with tile.TileContext(nc) as tc, \
         nc.allow_low_precision(reason="f32r datapath; rel-err budget 2e-2"):
        _emit(tc, nc, n_layers, d, phases)
    nc.compile()
    return nc


def _emit(tc, nc, n_layers, d, phases="ABCLD"):
    ctx = contextlib.ExitStack()

    # --- long-lived SBUF pools -------------------------------------------
    consts = ctx.enter_context(tc.tile_pool(name="consts", bufs=1))
    # unions: tags share a slot across phases of a layer (qT->nT, kT->res1,
    # v->res2); aT and x get their own.
    uni = ctx.enter_context(tc.tile_pool(name="uni", bufs=1))
    x_pool = ctx.enter_context(tc.tile_pool(name="xp", bufs=2))
    probs_pool = ctx.enter_context(tc.tile_pool(name="probs", bufs=3))
    stats_pool = ctx.enter_context(tc.tile_pool(name="stats", bufs=2))
    stats1_pool = ctx.enter_context(tc.tile_pool(name="stats1", bufs=1))
    wq_pool = ctx.enter_context(tc.tile_pool(name="wq", bufs=2))
    wp_pool = ctx.enter_context(tc.tile_pool(name="wp", bufs=1))
    wf_pool = ctx.enter_context(tc.tile_pool(name="wf", bufs=2))
    wr_pool = ctx.enter_context(tc.tile_pool(name="wr", bufs=3))
    gelu_pool = ctx.enter_context(tc.tile_pool(name="gelu", bufs=3))
    bias_pool = ctx.enter_context(tc.tile_pool(name="bias", bufs=1))

    # constants
    cn = {}
    cn["triu"] = consts.tile([128, 128], F32R, tag="triu", name="triu")       # triu[p, f] = 1 if p <= f
    nc.sync.dma_start(out=cn["triu"], in_=d["triu"])
    cn["ones1"] = consts.tile([1, 128], F32R, tag="ones1", name="ones1")        # full bcast lhsT (K=1)
    nc.sync.dma_start(out=cn["ones1"], in_=d["ones_row"])
    cn["ones64"] = cn["ones1"][:, 0:64]                          # head bcast lhsT (K=1)
    cn["ones_red"] = consts.tile([128, 2], F32R, tag="ones_red", name="ones_red")     # partition-sum lhsT (M=2)
    nc.sync.dma_start(out=cn["ones_red"], in_=d["ones_red"])
    cn["sel_den"] = consts.tile([128, 4, 4], F32R, tag="sel_den", name="sel_den")
    nc.sync.dma_start(out=cn["sel_den"], in_=d["sel_den"])
    cn["sel_bc"] = consts.tile([4, 4, 64], F32R, tag="sel_bc", name="sel_bc")
    nc.sync.dma_start(out=cn["sel_bc"], in_=d["sel_bc"])
    cn["eps"] = consts.tile([1, 1], F32, tag="eps", name="eps")
    nc.vector.memset(cn["eps"], EPS)

    pools = dict(uni=uni, x=x_pool, probs=probs_pool, stats=stats_pool,
                 stats1=stats1_pool,
                 wq=wq_pool, wp=wp_pool, wf=wf_pool, wr=wr_pool,
                 gelu=gelu_pool, bias=bias_pool)

    # residual stream x^T, [128, KC, S] (chunk-major)
    x_cur = x_pool.tile([128, KC, S], F32R, tag="x")
    nc.sync.dma_start(out=x_cur, in_=d["x0"].rearrange("(k p) s -> p k s", p=128))

    for l in range(n_layers):
        with nc.named_scope(f"layer{l}"):
            x_cur = _layer(tc, nc, l, x_cur, d, cn, pools, phases)

    nc.sync.dma_start(out=d["out"].rearrange("(k p) s -> p k s", p=128), in_=x_cur)
    ctx.close()


def _ld_bias(nc, pool, dram_ap, tag, width):
    t = pool.tile([128, width], F32, tag=tag, name=tag)
    nc.sync.dma_start(out=t, in_=dram_ap.rearrange("(c p) -> p c", p=128))
    return t


def _layer(tc, nc, l, x_cur, d, cn, pools, phases="ABCLD"):
    uni = pools["uni"]; stats_pool = pools["stats"]; bias_pool = pools["bias"]
    stats1_pool = pools["stats1"]

    # ---- biases / ln params for this layer ------------------------------
    bqkv_t = _ld_bias(nc, bias_pool, d["bqkv"][l], "bqkv", 3 * D // 128)
    bv_b = bias_pool.tile([128, D], F32, tag="bvb")      # V bias, row-bcast
    nc.sync.dma_start(out=bv_b, in_=d["bqkv"][l, 2 * D:3 * D].partition_broadcast(128))
    bproj_t = _ld_bias(nc, bias_pool, d["bproj"][l], "bproj", KC)
    g1_t = _ld_bias(nc, bias_pool, d["g1"][l], "g1", KC)
    b1_t = _ld_bias(nc, bias_pool, d["b1"][l], "b1", KC)
    bfc_t = _ld_bias(nc, bias_pool, d["bfc"][l], "bfc", KF)
    bpr_t = _ld_bias(nc, bias_pool, d["bpr"][l], "bpr", KC)
    g2_t = _ld_bias(nc, bias_pool, d["g2"][l], "g2", KC)
    b2_t = _ld_bias(nc, bias_pool, d["b2"][l], "b2", KC)

    # =====================================================================
    # Phase A: qkv.  q^T,k^T transposed [128, KC, S]; V natural [sk, h, dh].
    # wqkv streamed in 6 column-parts of 384 (parts 0-3: Q,K; 4-5: V).
    # =====================================================================
    qT = uni.tile([128, KC, S], F32R, tag="u_q")         # -> nT later
    kT = uni.tile([128, KC, S], F32R, tag="u_k")         # -> res1 later
    v_nat = uni.tile([128, SC, H, DH], F32R, tag="u_v")  # -> res2 later

    with tc.tile_pool(name="ps_qk", bufs=3, space="PSUM") as ps_qk, \
         tc.tile_pool(name="ps_v", bufs=2, space="PSUM") as ps_v:
        for p in range(4):                 # Q/K parts: columns [384p, 384p+384)
            wpart = pools["wq"].tile([128, KC, 384], F32R, tag="wqkv")
            nc.sync.dma_start(
                out=wpart,
                in_=d["wqkv"][l, :, 384 * p:384 * (p + 1)].rearrange(
                    "(k q) n -> q k n", q=128))
            for j in range(3):
                oc = 3 * p + j             # output chunk of qk^T, 0..11
                pt = ps_qk.tile([128, S], F32)
                for k in range(KC):
                    nc.tensor.matmul(pt, wpart[:, k, 128 * j:128 * (j + 1)],
                                     x_cur[:, k, :], start=(k == 0),
                                     stop=(k == KC - 1))
                dst = qT[:, oc, :] if oc < KC else kT[:, oc - KC, :]
                nc.vector.tensor_scalar(out=dst, in0=pt,
                                        scalar1=bqkv_t[:, oc:oc + 1],
                                        scalar2=None, op0=OP.add)
        for p in range(4, 6):              # V parts: v-features [384(p-4), +384)
            wpart = pools["wq"].tile([128, KC, 384], F32R, tag="wqkv")
            nc.sync.dma_start(
                out=wpart,
                in_=d["wqkv"][l, :, 384 * p:384 * (p + 1)].rearrange(
                    "(k q) n -> q k n", q=128))
            n0 = 384 * (p - 4)
            for sc in range(SC):
                pv = ps_v.tile([128, 384], F32, tag="pv")
                for k in range(KC):
                    nc.tensor.matmul(pv, x_cur[:, k, 128 * sc:128 * (sc + 1)],
                                     wpart[:, k, :], start=(k == 0),
                                     stop=(k == KC - 1))
                # v_nat[:, sc, h, :] = pv + bias_v for heads n0/64 .. n0/64+5
                h0 = n0 // DH
                nc.vector.tensor_tensor(
                    out=v_nat[:, sc, h0:h0 + 6, :],
                    in0=pv.rearrange("q (h e) -> q h e", e=DH),
                    in1=bv_b[:, n0:n0 + 384].rearrange("q (h e) -> q h e", e=DH),
                    op=OP.add)

    if "B" not in phases:
        return qT
    # =====================================================================
    # Phase B: attention, head by head.  scores^T chunks via K^T.T @ Q^T.
    # =====================================================================
    aT = uni.tile([128, KC, S], F32R, tag="u_a")
    G = 4                                         # heads per recip group
    with tc.tile_pool(name="ps_sc", bufs=2, space="PSUM") as ps_sc, \
         tc.tile_pool(name="ps_av", bufs=1, space="PSUM") as ps_av, \
         tc.tile_pool(name="ps_dn", bufs=1, space="PSUM") as ps_dn, \
         tc.tile_pool(name="ps_bc", bufs=1, space="PSUM") as ps_bc:
        for g in range(H // G):
            pden = ps_dn.tile([G, S], F32, tag="den")
            pavs = []
            for j in range(G):
                h = G * g + j
                hc, hh = h // 2, (h % 2) * 64
                probs = pools["probs"].tile([128, SC, S], F32R, tag="probs")
                for c in range(SC):
                    n0 = 128 * c                     # sq range [n0, S)
                    pt = ps_sc.tile([128, S], F32, tag="score")
                    nc.tensor.matmul(pt[:, 0:S - n0],
                                     kT[hh:hh + 64, hc, n0:n0 + 128],
                                     qT[hh:hh + 64, hc, n0:S],
                                     start=True, stop=True)
                    nc.scalar.activation(out=probs[:, c, n0:S],
                                         in_=pt[:, 0:S - n0],
                                         func=AF.Exp, scale=0.125)
                    nc.vector.tensor_tensor(out=probs[:, c, n0:n0 + 128],
                                            in0=probs[:, c, n0:n0 + 128],
                                            in1=cn["triu"], op=OP.mult)
                # denominator rows, one-hot selected into pden row j
                for c in range(SC):
                    n0 = 128 * c
                    nc.tensor.matmul(pden[:, n0:S], cn["sel_den"][:, j, :],
                                     probs[:, c, n0:S],
                                     start=(j == 0 and c == 0),
                                     stop=(j == G - 1 and c == SC - 1),
                                     skip_group_check=True)
                # av^T [64, S] accumulated over sk chunks (suffix scheme)
                pav = ps_av.tile([128, S], F32, tag=f"av{j}", name=f"pav{j}")
                pavs.append(pav)
                for c in range(SC):
                    n0 = 128 * c
                    nc.tensor.matmul(pav[0:64, n0:S], v_nat[:, c, h, :],
                                     probs[:, c, n0:S],
                                     start=(c == 0), stop=(c == SC - 1),
                                     skip_group_check=True)
            recip32 = pools["stats1"].tile([G, S], F32, tag="recip32")
            nc.vector.reciprocal_approx_fast(out=recip32, in_=pden[0:G, :])
            recip_r = pools["stats1"].tile([G, S], F32R, tag="recipr")
            nc.vector.tensor_copy(out=recip_r, in_=recip32)
            for j in range(G):
                h = G * g + j
                hc, hh = h // 2, (h % 2) * 64
                pbc = ps_bc.tile([64, S], F32, tag="bc")
                nc.tensor.matmul(pbc, cn["sel_bc"][:, j, :], recip_r,
                                 start=True, stop=True)
                bc_s = stats_pool.tile([64, S], F32, tag="bc_s")
                nc.vector.tensor_copy(out=bc_s, in_=pbc)
                if hh == 0:
                    nc.vector.tensor_tensor(out=aT[0:64, hc, :],
                                            in0=pavs[j][0:64, :],
                                            in1=bc_s, op=OP.mult)
                else:
                    av_s = pools["stats1"].tile([64, S], F32R, tag="av_s")
                    nc.vector.tensor_tensor(out=av_s, in0=pavs[j][0:64, :],
                                            in1=bc_s, op=OP.mult)
                    nc.sync.dma_start(out=aT[64:128, hc, :], in_=av_s)

    if "C" not in phases:
        return aT
    # =====================================================================
    # Phase C: attn out proj + residual + LN1
    # =====================================================================
    wproj_t = pools["wp"].tile([128, KC, D], F32R, tag="wproj")
    nc.sync.dma_start(out=wproj_t,
                      in_=d["wproj"][l].rearrange("(k p) n -> p k n", p=128))
    res1 = uni.tile([128, KC, S], F32R, tag="u_k")       # reuses kT slot
    with tc.tile_pool(name="ps_pj", bufs=3, space="PSUM") as ps_pj:
        for oc in range(KC):
            pt = ps_pj.tile([128, S], F32)
            for k in range(KC):
                nc.tensor.matmul(pt, wproj_t[:, k, 128 * oc:128 * (oc + 1)],
                                 aT[:, k, :], start=(k == 0), stop=(k == KC - 1))
            nc.vector.tensor_scalar(out=pt, in0=pt, scalar1=bproj_t[:, oc:oc + 1],
                                    scalar2=None, op0=OP.add)
            nc.vector.tensor_tensor(out=res1[:, oc, :], in0=pt,
                                    in1=x_cur[:, oc, :], op=OP.add)

    if "L" not in phases:
        return res1
    nT = uni.tile([128, KC, S], F32R, tag="u_q")         # reuses qT slot
    _layernorm(tc, nc, res1, nT, g1_t, b1_t, cn, stats_pool, stats1_pool, "ln1")

    if "D" not in phases:
        return nT
    # =====================================================================
    # Phase D: fused fc -> gelu -> pr (+ residual), k-outer over DF chunks.
    # wfc streamed in 6 column-parts of 512 (4 kf each); wpr in row-slices.
    # =====================================================================
    res2 = uni.tile([128, KC, S], F32R, tag="u_v")       # reuses v_nat slot
    with tc.tile_pool(name="ps_pr", bufs=1, space="PSUM") as ps_pr, \
         tc.tile_pool(name="ps_fc", bufs=2, space="PSUM") as ps_fc:
        pr_acc = [ps_pr.tile([128, S], F32, tag=f"pr{oc}", name=f"pr{oc}") for oc in range(KC)]
        for part in range(6):
            wfc_p = pools["wf"].tile([128, KC, 512], F32R, tag="wfc")
            nc.sync.dma_start(
                out=wfc_p,
                in_=d["wfc"][l, :, 512 * part:512 * (part + 1)].rearrange(
                    "(k q) n -> q k n", q=128))
            for j in range(4):
                kf = 4 * part + j
                wpr_k = pools["wr"].tile([128, D], F32R, tag="wprk")
                nc.sync.dma_start(out=wpr_k,
                                  in_=d["wpr"][l, 128 * kf:128 * (kf + 1), :])
                pf = ps_fc.tile([128, S], F32)
                for k in range(KC):
                    nc.tensor.matmul(pf, wfc_p[:, k, 128 * j:128 * (j + 1)],
                                     nT[:, k, :], start=(k == 0),
                                     stop=(k == KC - 1))
                gk = pools["gelu"].tile([128, S], F32R, tag="gk")
                nc.scalar.activation(out=gk, in_=pf, func=AF.Gelu_apprx_tanh,
                                     bias=bfc_t[:, kf:kf + 1], scale=1.0)
                for oc in range(KC):
                    nc.tensor.matmul(pr_acc[oc],
                                     wpr_k[:, 128 * oc:128 * (oc + 1)],
                                     gk, start=(kf == 0), stop=(kf == KF - 1))
        for oc in range(KC):
            nc.vector.tensor_scalar(out=pr_acc[oc], in0=pr_acc[oc],
                                    scalar1=bpr_t[:, oc:oc + 1],
                                    scalar2=None, op0=OP.add)
            nc.vector.tensor_tensor(out=res2[:, oc, :], in0=pr_acc[oc],
                                    in1=nT[:, oc, :], op=OP.add)

    x_next = pools["x"].tile([128, KC, S], F32R, tag="x")
    _layernorm(tc, nc, res2, x_next, g2_t, b2_t, cn, stats_pool, stats1_pool, "ln2")
    return x_next


def _layernorm(tc, nc, src, dst, g_t, b_t, cn, stats_pool, stats1_pool, tag):
    """LN over the partition (feature) axis of src [128, KC, S] -> dst."""
    with tc.tile_pool(name=f"ps_{tag}", bufs=1, space="PSUM") as ps:
        psum = ps.tile([2, S], F32, tag="s0")        # row 0: sum(x)
        psq = ps.tile([2, S], F32, tag="s1")         # row 0: sum(x^2)
        for k in range(KC):
            sq = stats_pool.tile([128, S], F32R, tag="lnsq")
            nc.scalar.activation(out=sq, in_=src[:, k, :], func=AF.Square)
            nc.tensor.matmul(psum, cn["ones_red"], src[:, k, :],
                             start=(k == 0), stop=(k == KC - 1))
            nc.tensor.matmul(psq, cn["ones_red"], sq,
                             start=(k == 0), stop=(k == KC - 1))
        mu = stats1_pool.tile([1, S], F32R, tag="mu")
        rsd = stats1_pool.tile([1, S], F32R, tag="rsd")
        var = stats1_pool.tile([1, S], F32, tag="var")
        nc.vector.tensor_scalar(out=mu, in0=psum[0:1, :], scalar1=1.0 / D,
                                scalar2=None, op0=OP.mult)
        nc.vector.tensor_tensor(out=var, in0=mu, in1=mu, op=OP.mult)
        nc.vector.scalar_tensor_tensor(out=var, in0=psq[0:1, :], scalar=1.0 / D,
                                       in1=var, op0=OP.mult, op1=OP.subtract)
        nc.scalar.activation(out=var, in_=var, func=AF.Sqrt, bias=cn["eps"])
        rsd32 = stats1_pool.tile([1, S], F32, tag="rsd32")
        nc.vector.reciprocal_approx_fast(out=rsd32, in_=var)
        nc.vector.tensor_copy(out=rsd, in_=rsd32)
        pmu = ps.tile([128, S], F32, tag="bmu")
        prs = ps.tile([128, S], F32, tag="brs")
        nc.tensor.matmul(pmu, cn["ones1"], mu, start=True, stop=True)
        nc.tensor.matmul(prs, cn["ones1"], rsd, start=True, stop=True)
        for k in range(KC):
            t = stats_pool.tile([128, S], F32, tag="lnt")
            nc.vector.tensor_tensor(out=t, in0=src[:, k, :], in1=pmu,
                                    op=OP.subtract)
            nc.vector.tensor_tensor(out=t, in0=t, in1=prs, op=OP.mult)
            nc.vector.tensor_scalar(out=dst[:, k, :], in0=t,
                                    scalar1=g_t[:, k:k + 1],
                                    scalar2=b_t[:, k:k + 1],
                                    op0=OP.mult, op1=OP.add)


# =========================================================================
# Host side
# =========================================================================
_CACHE = {}


def _get_program():
    if "nc" not in _CACHE:
        _install_ntff_hook()
        _CACHE["nc"] = build_program(L)
    return _CACHE["nc"]


def make_in_maps(inputs, n_layers=L):
    tokens = np.asarray(inputs["tokens"])
    we = np.asarray(inputs["we"], dtype=np.float32)
    pos = we[V:V + S]                                  # [S, D]
    triu = np.triu(np.ones((128, 128), dtype=np.float32))

    def f32(name):
        return np.ascontiguousarray(np.asarray(inputs[name])[:n_layers],
                                    dtype=np.float32)

    shared = {k: f32(k) for k in ["wqkv", "bqkv", "wproj", "bproj", "g1", "b1",
                                  "wfc", "bfc", "wpr", "bpr", "g2", "b2"]}
    shared["triu"] = triu
    shared["ones_row"] = np.ones((1, 128), dtype=np.float32)
    onesred = np.zeros((128, 2), dtype=np.float32); onesred[:, 0] = 1.0
    shared["ones_red"] = onesred
    sel_den = np.zeros((128, 4, 4), dtype=np.float32)
    for j in range(4):
        sel_den[:, j, j] = 1.0
    shared["sel_den"] = sel_den
    sel_bc = np.zeros((4, 4, 64), dtype=np.float32)
    for j in range(4):
        sel_bc[j, j, :] = 1.0
    shared["sel_bc"] = sel_bc
    in_maps = []
    for b in range(N_CORES):
        x0 = we[tokens[b]] + pos                       # [S, D]
        m = dict(shared)
        m["x0T"] = np.ascontiguousarray(x0.T, dtype=np.float32)
        in_maps.append(m)
    return in_maps


def run(inputs, trace=False):
    nc = _get_program()
    in_maps = make_in_maps(inputs)
    res = bass_utils.run_bass_kernel_spmd(nc, in_maps,
                                          core_ids=list(range(N_CORES)),
                                          trace=trace)
    outs = np.stack([res.results[b]["out"].T for b in range(N_CORES)])
    return outs.astype(np.float32), res


def kernel(**inputs):
    out, _ = run(inputs, trace=False)
    return out


# revision 13
# speedup vs baseline: 1.1601x; 1.0098x over previous
"""Bass/Trainium2 kernel for a 12-layer GPT-style transformer (nn_BERT).

Strategy: data-parallel over batch (B=8 -> 1 sequence per NeuronCore).
Each core runs all 12 layers on x^T [D=768, S=512] in "transposed"
activation layout (feature dim on partitions), f32r matmul datapath.

kernel(**inputs) takes the FULL unsharded inputs (as produced by
reference.setup_inputs()) and returns the full [8, 512, 768] output.
"""
import contextlib
import os
import sys
import types

sys.path.insert(0, "/opt/trn_rl_repo")
os.environ.setdefault("JAX_PLATFORMS", "axon")

import numpy as np

import concourse.bass as bass
import concourse.mybir as mybir
import concourse.tile as tile
from concourse import bacc
from concourse import bass_utils

F32 = mybir.dt.float32
F32R = mybir.dt.float32r
AF = mybir.ActivationFunctionType
OP = mybir.AluOpType

B, S, D, H, L, V = 8, 512, 768, 12, 12, 40478
DH = D // H            # 64
DF = 4 * D             # 3072
KC = D // 128          # 6 chunks of the model dim
KF = DF // 128         # 24 chunks of the ffn dim
SC = S // 128          # 4 chunks of the sequence
EPS = 1e-5

N_CORES = 8


def _install_ntff_hook():
    """Register the axon NTFF profiling hook that this image's antenv lacks."""
    if "antenv.axon_hooks" in sys.modules:
        return
    try:
        mod = types.ModuleType("antenv.axon_hooks")
        _h = [None]
        mod.set_axon_ntff_profile_hook = lambda h: _h.__setitem__(0, h)
        mod.get_axon_ntff_profile_hook = lambda: _h[0]
        sys.modules["antenv.axon_hooks"] = mod
        import antenv

        antenv.axon_hooks = mod
        if "/root/.axon_site" not in sys.path:
            sys.path.insert(0, "/root/.axon_site")
        from trn_agent_boot.trn_boot import _ntff_profile_via_ctypes

        mod.set_axon_ntff_profile_hook(
            _ntff_profile_via_ctypes("/opt/axon/libaxon_pjrt.so")
        )
    except Exception:
        pass


def build_program(n_layers=L, phases="ABCLD"):
    nc = bacc.Bacc("TRN2", target_bir_lowering=False, debug=False,
                   num_devices=N_CORES)

    d = {}
    d["x0"] = nc.dram_tensor("x0T", (D, S), F32R, kind="ExternalInput").ap()
    d["wqkv"] = nc.dram_tensor("wqkv", (n_layers, D, 3 * D), F32R, kind="ExternalInput").ap()
    d["bqkv"] = nc.dram_tensor("bqkv", (n_layers, 3 * D), F32, kind="ExternalInput").ap()
    d["wproj"] = nc.dram_tensor("wproj", (n_layers, D, D), F32R, kind="ExternalInput").ap()
    d["bproj"] = nc.dram_tensor("bproj", (n_layers, D), F32, kind="ExternalInput").ap()
    d["g1"] = nc.dram_tensor("g1", (n_layers, D), F32, kind="ExternalInput").ap()
    d["b1"] = nc.dram_tensor("b1", (n_layers, D), F32, kind="ExternalInput").ap()
    d["wfc"] = nc.dram_tensor("wfc", (n_layers, D, DF), F32R, kind="ExternalInput").ap()
    d["bfc"] = nc.dram_tensor("bfc", (n_layers, DF), F32, kind="ExternalInput").ap()
    d["wpr"] = nc.dram_tensor("wpr", (n_layers, DF, D), F32R, kind="ExternalInput").ap()
    d["bpr"] = nc.dram_tensor("bpr", (n_layers, D), F32, kind="ExternalInput").ap()
    d["g2"] = nc.dram_tensor("g2", (n_layers, D), F32, kind="ExternalInput").ap()
    d["b2"] = nc.dram_tensor("b2", (n_layers, D), F32, kind="ExternalInput").ap()
    d["triu"] = nc.dram_tensor("triu", (128, 128), F32R, kind="ExternalInput").ap()
    d["ones_row"] = nc.dram_tensor("ones_row", (1, 128), F32R, kind="ExternalInput").ap()
    d["ones_red"] = nc.dram_tensor("ones_red", (128, 2), F32R, kind="ExternalInput").ap()
    d["sel_den"] = nc.dram_tensor("sel_den", (128, 4, 4), F32R, kind="ExternalInput").ap()
    d["sel_bc2"] = nc.dram_tensor("sel_bc2", (4, 2, 128), F32R, kind="ExternalInput").ap()
    d["ones2d"] = nc.dram_tensor("ones2d", (128, 128), F32R, kind="ExternalInput").ap()
    d["out"] = nc.dram_tensor("out", (D, S), F32R, kind="ExternalOutput").ap()

    with tile.TileContext(nc) as tc, \
         nc.allow_low_precision(reason="f32r datapath; rel-err budget 2e-2"):
        _emit(tc, nc, n_layers, d, phases)
    nc.compile()
    return nc


def _emit(tc, nc, n_layers, d, phases="ABCLD"):
    ctx = contextlib.ExitStack()

    # --- long-lived SBUF pools -------------------------------------------
    consts = ctx.enter_context(tc.tile_pool(name="consts", bufs=1))
    # unions: tags share a slot across phases of a layer (qT->nT, kT->res1,
    # v->res2); aT and x get their own.
    uni = ctx.enter_context(tc.tile_pool(name="uni", bufs=1))
    x_pool = ctx.enter_context(tc.tile_pool(name="xp", bufs=2))
    probs_pool = ctx.enter_context(tc.tile_pool(name="probs", bufs=3))
    stats_pool = ctx.enter_context(tc.tile_pool(name="stats", bufs=2))
    stats1_pool = ctx.enter_context(tc.tile_pool(name="stats1", bufs=1))
    wq_pool = ctx.enter_context(tc.tile_pool(name="wq", bufs=2))
    wp_pool = ctx.enter_context(tc.tile_pool(name="wp", bufs=1))
    wf_pool = ctx.enter_context(tc.tile_pool(name="wf", bufs=2))
    wr_pool = ctx.enter_context(tc.tile_pool(name="wr", bufs=3))
    gelu_pool = ctx.enter_context(tc.tile_pool(name="gelu", bufs=3))
    bias_pool = ctx.enter_context(tc.tile_pool(name="bias", bufs=1))

    # constants
    cn = {}
    cn["triu"] = consts.tile([128, 128], F32R, tag="triu", name="triu")       # triu[p, f] = 1 if p <= f
    nc.sync.dma_start(out=cn["triu"], in_=d["triu"])
    cn["ones1"] = consts.tile([1, 128], F32R, tag="ones1", name="ones1")        # full bcast lhsT (K=1)
    nc.sync.dma_start(out=cn["ones1"], in_=d["ones_row"])
    cn["ones_red"] = consts.tile([128, 2], F32R, tag="ones_red", name="ones_red")     # partition-sum lhsT (M=2)
    nc.sync.dma_start(out=cn["ones_red"], in_=d["ones_red"])
    cn["sel_den"] = consts.tile([128, 4, 4], F32R, tag="sel_den", name="sel_den")
    nc.sync.dma_start(out=cn["sel_den"], in_=d["sel_den"])
    cn["sel_bc2"] = consts.tile([4, 2, 128], F32R, tag="sel_bc2", name="sel_bc2")
    nc.sync.dma_start(out=cn["sel_bc2"], in_=d["sel_bc2"])
    cn["ones2d"] = consts.tile([128, 128], F32R, tag="ones2d", name="ones2d")
    nc.sync.dma_start(out=cn["ones2d"], in_=d["ones2d"])
    cn["eps"] = consts.tile([1, 1], F32, tag="eps", name="eps")
    nc.vector.memset(cn["eps"], EPS)

    pools = dict(uni=uni, x=x_pool, probs=probs_pool, stats=stats_pool,
                 stats1=stats1_pool,
                 wq=wq_pool, wp=wp_pool, wf=wf_pool, wr=wr_pool,
                 gelu=gelu_pool, bias=bias_pool)

    # residual stream x^T, [128, KC, S] (chunk-major)
    x_cur = x_pool.tile([128, KC, S], F32R, tag="x")
    nc.sync.dma_start(out=x_cur, in_=d["x0"].rearrange("(k p) s -> p k s", p=128))

    for l in range(n_layers):
        with nc.named_scope(f"layer{l}"):
            x_cur = _layer(tc, nc, l, x_cur, d, cn, pools, phases)

    nc.sync.dma_start(out=d["out"].rearrange("(k p) s -> p k s", p=128), in_=x_cur)
    ctx.close()


def _ld_bias(nc, pool, dram_ap, tag, width):
    t = pool.tile([128, width], F32, tag=tag, name=tag)
    nc.sync.dma_start(out=t, in_=dram_ap.rearrange("(c p) -> p c", p=128))
    return t


def _layer(tc, nc, l, x_cur, d, cn, pools, phases="ABCLD"):
    uni = pools["uni"]; stats_pool = pools["stats"]; bias_pool = pools["bias"]
    stats1_pool = pools["stats1"]

    # ---- biases / ln params for this layer ------------------------------
    bqkv_t = _ld_bias(nc, bias_pool, d["bqkv"][l], "bqkv", 3 * D // 128)
    bv_b = bias_pool.tile([128, D], F32, tag="bvb")      # V bias, row-bcast
    nc.sync.dma_start(out=bv_b, in_=d["bqkv"][l, 2 * D:3 * D].partition_broadcast(128))
    bproj_t = _ld_bias(nc, bias_pool, d["bproj"][l], "bproj", KC)
    g1_t = _ld_bias(nc, bias_pool, d["g1"][l], "g1", KC)
    b1_t = _ld_bias(nc, bias_pool, d["b1"][l], "b1", KC)
    bfc_t = _ld_bias(nc, bias_pool, d["bfc"][l], "bfc", KF)
    bpr_t = _ld_bias(nc, bias_pool, d["bpr"][l], "bpr", KC)
    g2_t = _ld_bias(nc, bias_pool, d["g2"][l], "g2", KC)
    b2_t = _ld_bias(nc, bias_pool, d["b2"][l], "b2", KC)

    # =====================================================================
    # Phase A: qkv.  q^T,k^T transposed [128, KC, S]; V natural [sk, h, dh].
    # wqkv streamed in 6 column-parts of 384 (parts 0-3: Q,K; 4-5: V).
    # =====================================================================
    qT = uni.tile([128, KC, S], F32R, tag="u_q")         # -> nT later
    kT = uni.tile([128, KC, S], F32R, tag="u_k")         # -> res1 later
    v_nat = uni.tile([128, SC, H, DH], F32R, tag="u_v")  # -> res2 later

    with tc.tile_pool(name="ps_qk", bufs=3, space="PSUM") as ps_qk, \
         tc.tile_pool(name="ps_v", bufs=2, space="PSUM") as ps_v:
        for p in range(4):                 # Q/K parts: columns [384p, 384p+384)
            wpart = pools["wq"].tile([128, KC, 384], F32R, tag="wqkv")
            nc.sync.dma_start(
                out=wpart,
                in_=d["wqkv"][l, :, 384 * p:384 * (p + 1)].rearrange(
                    "(k q) n -> q k n", q=128))
            for j in range(3):
                oc = 3 * p + j             # output chunk of qk^T, 0..11
                pt = ps_qk.tile([128, S], F32)
                for k in range(KC):
                    nc.tensor.matmul(pt, wpart[:, k, 128 * j:128 * (j + 1)],
                                     x_cur[:, k, :], start=(k == 0),
                                     stop=(k == KC - 1))
                dst = qT[:, oc, :] if oc < KC else kT[:, oc - KC, :]
                nc.vector.tensor_scalar(out=dst, in0=pt,
                                        scalar1=bqkv_t[:, oc:oc + 1],
                                        scalar2=None, op0=OP.add)
        for p in range(4, 6):              # V parts: v-features [384(p-4), +384)
            wpart = pools["wq"].tile([128, KC, 384], F32R, tag="wqkv")
            nc.sync.dma_start(
                out=wpart,
                in_=d["wqkv"][l, :, 384 * p:384 * (p + 1)].rearrange(
                    "(k q) n -> q k n", q=128))
            n0 = 384 * (p - 4)
            for sc in range(SC):
                pv = ps_v.tile([128, 384], F32, tag="pv")
                for k in range(KC):
                    nc.tensor.matmul(pv, x_cur[:, k, 128 * sc:128 * (sc + 1)],
                                     wpart[:, k, :], start=(k == 0),
                                     stop=(k == KC - 1))
                # v_nat[:, sc, h, :] = pv + bias_v for heads n0/64 .. n0/64+5
                h0 = n0 // DH
                nc.vector.tensor_tensor(
                    out=v_nat[:, sc, h0:h0 + 6, :],
                    in0=pv.rearrange("q (h e) -> q h e", e=DH),
                    in1=bv_b[:, n0:n0 + 384].rearrange("q (h e) -> q h e", e=DH),
                    op=OP.add)

    if "B" not in phases:
        return qT
    # =====================================================================
    # Phase B: attention, head by head.  scores^T chunks via K^T.T @ Q^T.
    # =====================================================================
    aT = uni.tile([128, KC, S], F32R, tag="u_a")
    G = 4                                         # heads per recip group
    with tc.tile_pool(name="ps_sc", bufs=2, space="PSUM") as ps_sc, \
         tc.tile_pool(name="ps_av", bufs=1, space="PSUM") as ps_av, \
         tc.tile_pool(name="ps_dn", bufs=1, space="PSUM") as ps_dn, \
         tc.tile_pool(name="ps_bc", bufs=1, space="PSUM") as ps_bc:
        for g in range(H // G):
            pden = ps_dn.tile([G, S], F32, tag="den")
            pavs = []
            for j in range(G):
                h = G * g + j
                hc, hh = h // 2, (h % 2) * 64
                probs = pools["probs"].tile([128, SC, S], F32R, tag="probs")
                for c in range(SC):
                    n0 = 128 * c if c < SC - 1 else 256   # widened chunk 3
                    pt = ps_sc.tile([128, S], F32, tag="score")
                    nc.tensor.matmul(pt[:, 0:S - n0],
                                     kT[hh:hh + 64, hc, 128 * c:128 * c + 128],
                                     qT[hh:hh + 64, hc, n0:S],
                                     start=True, stop=True)
                    nc.scalar.activation(out=probs[:, c, n0:S],
                                         in_=pt[:, 0:S - n0],
                                         func=AF.Exp, scale=0.125)
                    if c == SC - 1:
                        # zero the fully-masked sq block [256, 384)
                        nc.vector.tensor_scalar(out=probs[:, c, 256:384],
                                                in0=probs[:, c, 256:384],
                                                scalar1=0.0, scalar2=None,
                                                op0=OP.mult)
                    nc.vector.tensor_tensor(
                        out=probs[:, c, 128 * c:128 * c + 128],
                        in0=probs[:, c, 128 * c:128 * c + 128],
                        in1=cn["triu"], op=OP.mult)
                # denominator rows, one-hot selected into pden row j
                for c in range(SC):
                    n0 = 128 * c if c < SC - 1 else 256
                    nc.tensor.matmul(pden[:, n0:S], cn["sel_den"][:, j, :],
                                     probs[:, c, n0:S],
                                     start=(j == 0 and c == 0),
                                     stop=(j == G - 1 and c == SC - 1),
                                     skip_group_check=True)
                # av^T accumulated over sk chunks; even head -> rows 0:64 of
                # its own psum, odd head -> rows 64:128 (pair-view lhsT).
                if hh == 0:
                    pav = ps_av.tile([64, S], F32, tag=f"av_e{j % 2}",
                                     name=f"pav_e{j % 2}")
                    lhsT = v_nat[:, :, h, :]
                else:
                    pav = ps_av.tile([128, S], F32, tag=f"av_o{j % 2}",
                                     name=f"pav_o{j % 2}")
                    lhsT = v_nat[:, :, h - 1:h + 1, :]
                pavs.append(pav)
                for c in range(SC):
                    n0 = 128 * c if c < SC - 1 else 256
                    if hh == 0:
                        lt = lhsT[:, c, :]
                        dst = pav[0:64, n0:S]
                    else:
                        lt = lhsT[:, c, :, :].rearrange("p h e -> p (h e)")
                        dst = pav[0:128, n0:S]
                    nc.tensor.matmul(dst, lt, probs[:, c, n0:S],
                                     start=(c == 0), stop=(c == SC - 1),
                                     skip_group_check=True)
            recip32 = pools["stats1"].tile([G, S], F32, tag="recip32")
            nc.vector.reciprocal_approx_fast(out=recip32, in_=pden[0:G, :])
            recip_r = pools["stats1"].tile([G, S], F32R, tag="recipr")
            nc.vector.tensor_copy(out=recip_r, in_=recip32)
            for q in range(2):                     # head pairs in this group
                hc = 2 * g + q
                pbc = ps_bc.tile([128, S], F32, tag="bc")
                nc.tensor.matmul(pbc, cn["sel_bc2"][:, q, :], recip_r,
                                 start=True, stop=True)
                bc_s = stats_pool.tile([128, S], F32, tag="bc_s")
                nc.vector.tensor_copy(out=bc_s, in_=pbc)
                nc.vector.tensor_tensor(out=aT[0:64, hc, :],
                                        in0=pavs[2 * q][0:64, :],
                                        in1=bc_s[0:64, :], op=OP.mult)
                nc.vector.tensor_tensor(out=aT[64:128, hc, :],
                                        in0=pavs[2 * q + 1][64:128, :],
                                        in1=bc_s[64:128, :], op=OP.mult)

    # =====================================================================
    # Phase C: attn out proj + residual + LN1
    # =====================================================================
    wproj_t = pools["wp"].tile([128, KC, D], F32R, tag="wproj")
    nc.sync.dma_start(out=wproj_t,
                      in_=d["wproj"][l].rearrange("(k p) n -> p k n", p=128))
    res1 = uni.tile([128, KC, S], F32R, tag="u_k")       # reuses kT slot
    with tc.tile_pool(name="ps_pj", bufs=3, space="PSUM") as ps_pj:
        for oc in range(KC):
            pt = ps_pj.tile([128, S], F32)
            for k in range(KC):
                nc.tensor.matmul(pt, wproj_t[:, k, 128 * oc:128 * (oc + 1)],
                                 aT[:, k, :], start=(k == 0), stop=(k == KC - 1))
            nc.vector.tensor_scalar(out=pt, in0=pt, scalar1=bproj_t[:, oc:oc + 1],
                                    scalar2=None, op0=OP.add)
            nc.vector.tensor_tensor(out=res1[:, oc, :], in0=pt,
                                    in1=x_cur[:, oc, :], op=OP.add)

    if "L" not in phases:
        return res1
    nT = uni.tile([128, KC, S], F32R, tag="u_q")         # reuses qT slot
    _layernorm(tc, nc, res1, nT, g1_t, b1_t, cn, stats_pool, stats1_pool, "ln1")

    if "D" not in phases:
        return nT
    # =====================================================================
    # Phase D: fused fc -> gelu -> pr (+ residual), k-outer over DF chunks.
    # wfc streamed in 6 column-parts of 512 (4 kf each); wpr in row-slices.
    # =====================================================================
    res2 = uni.tile([128, KC, S], F32R, tag="u_v")       # reuses v_nat slot
    with tc.tile_pool(name="ps_pr", bufs=1, space="PSUM") as ps_pr, \
         tc.tile_pool(name="ps_fc", bufs=2, space="PSUM") as ps_fc:
        pr_acc = [ps_pr.tile([128, S], F32, tag=f"pr{oc}", name=f"pr{oc}") for oc in range(KC)]
        for part in range(6):
            wfc_p = pools["wf"].tile([128, KC, 512], F32R, tag="wfc")
            nc.sync.dma_start(
                out=wfc_p,
                in_=d["wfc"][l, :, 512 * part:512 * (part + 1)].rearrange(
                    "(k q) n -> q k n", q=128))
            for j in range(4):
                kf = 4 * part + j
                wpr_k = pools["wr"].tile([128, D], F32R, tag="wprk")
                nc.sync.dma_start(out=wpr_k,
                                  in_=d["wpr"][l, 128 * kf:128 * (kf + 1), :])
                pf = ps_fc.tile([128, S], F32)
                for k in range(KC):
                    nc.tensor.matmul(pf, wfc_p[:, k, 128 * j:128 * (j + 1)],
                                     nT[:, k, :], start=(k == 0),
                                     stop=(k == KC - 1))
                gk = pools["gelu"].tile([128, S], F32R, tag="gk")
                nc.scalar.activation(out=gk, in_=pf, func=AF.Gelu_apprx_tanh,
                                     bias=bfc_t[:, kf:kf + 1], scale=1.0)
                for oc in range(KC):
                    nc.tensor.matmul(pr_acc[oc],
                                     wpr_k[:, 128 * oc:128 * (oc + 1)],
                                     gk, start=(kf == 0), stop=(kf == KF - 1))
        for oc in range(KC):
            nc.vector.tensor_scalar(out=pr_acc[oc], in0=pr_acc[oc],
                                    scalar1=bpr_t[:, oc:oc + 1],
                                    scalar2=None, op0=OP.add)
            nc.vector.tensor_tensor(out=res2[:, oc, :], in0=pr_acc[oc],
                                    in1=nT[:, oc, :], op=OP.add)

    x_next = pools["x"].tile([128, KC, S], F32R, tag="x")
    _layernorm(tc, nc, res2, x_next, g2_t, b2_t, cn, stats_pool, stats1_pool, "ln2")
    return x_next


def _layernorm(tc, nc, src, dst, g_t, b_t, cn, stats_pool, stats1_pool, tag):
    """LN over the partition (feature) axis of src [128, KC, S] -> dst."""
    with tc.tile_pool(name=f"ps_{tag}", bufs=1, space="PSUM") as ps:
        psums = ps.tile([128, S], F32, tag="bsum")   # every row = sum(x)
        psq = ps.tile([2, S], F32, tag="s1")         # row 0: sum(x^2)
        for k in range(KC):
            sq = stats_pool.tile([128, S], F32R, tag="lnsq")
            nc.scalar.activation(out=sq, in_=src[:, k, :], func=AF.Square)
            nc.tensor.matmul(psums, cn["ones2d"], src[:, k, :],
                             start=(k == 0), stop=(k == KC - 1))
            nc.tensor.matmul(psq, cn["ones_red"], sq,
                             start=(k == 0), stop=(k == KC - 1))
        mu1 = stats1_pool.tile([1, S], F32, tag="mu1")
        var = stats1_pool.tile([1, S], F32, tag="var")
        rsd = stats1_pool.tile([1, S], F32R, tag="rsd")
        nc.vector.tensor_scalar(out=mu1, in0=psums[0:1, :], scalar1=1.0 / D,
                                scalar2=None, op0=OP.mult)
        nc.vector.tensor_tensor(out=var, in0=mu1, in1=mu1, op=OP.mult)
        nc.vector.scalar_tensor_tensor(out=var, in0=psq[0:1, :], scalar=1.0 / D,
                                       in1=var, op0=OP.mult, op1=OP.subtract)
        nc.scalar.activation(out=var, in_=var, func=AF.Sqrt, bias=cn["eps"])
        rsd32 = stats1_pool.tile([1, S], F32, tag="rsd32")
        nc.vector.reciprocal_approx_fast(out=rsd32, in_=var)
        nc.vector.tensor_copy(out=rsd, in_=rsd32)
        prs = ps.tile([128, S], F32, tag="brs")
        nc.tensor.matmul(prs, cn["ones1"], rsd, start=True, stop=True)
        for k in range(KC):
            t = stats_pool.tile([128, S], F32, tag="lnt")
            # t = src - mean  (mean folded from the broadcast sums)
            nc.vector.scalar_tensor_tensor(out=t, in0=psums, scalar=-1.0 / D,
                                           in1=src[:, k, :], op0=OP.mult,
                                           op1=OP.add)
            nc.vector.tensor_tensor(out=t, in0=t, in1=prs, op=OP.mult)
            nc.vector.tensor_scalar(out=dst[:, k, :], in0=t,
                                    scalar1=g_t[:, k:k + 1],
                                    scalar2=b_t[:, k:k + 1],
                                    op0=OP.mult, op1=OP.add)



# =========================================================================
# Host side
# =========================================================================
_CACHE = {}


def _get_program():
    if "nc" not in _CACHE:
        _install_ntff_hook()
        _CACHE["nc"] = build_program(L)
    return _CACHE["nc"]


def make_in_maps(inputs, n_layers=L):
    tokens = np.asarray(inputs["tokens"])
    we = np.asarray(inputs["we"], dtype=np.float32)
    pos = we[V:V + S]                                  # [S, D]
    triu = np.triu(np.ones((128, 128), dtype=np.float32))

    def f32(name):
        return np.ascontiguousarray(np.asarray(inputs[name])[:n_layers],
                                    dtype=np.float32)

    shared = {k: f32(k) for k in ["wqkv", "bqkv", "wproj", "bproj", "g1", "b1",
                                  "wfc", "bfc", "wpr", "bpr", "g2", "b2"]}
    shared["triu"] = triu
    shared["ones_row"] = np.ones((1, 128), dtype=np.float32)
    onesred = np.zeros((128, 2), dtype=np.float32); onesred[:, 0] = 1.0
    shared["ones_red"] = onesred
    sel_den = np.zeros((128, 4, 4), dtype=np.float32)
    for j in range(4):
        sel_den[:, j, j] = 1.0
    shared["sel_den"] = sel_den
    sel_bc2 = np.zeros((4, 2, 128), dtype=np.float32)
    for q in range(2):
        sel_bc2[2 * q, q, 0:64] = 1.0
        sel_bc2[2 * q + 1, q, 64:128] = 1.0
    shared["sel_bc2"] = sel_bc2
    shared["ones2d"] = np.ones((128, 128), dtype=np.float32)
    in_maps = []
    for b in range(N_CORES):
        x0 = we[tokens[b]] + pos                       # [S, D]
        m = dict(shared)
        m["x0T"] = np.ascontiguousarray(x0.T, dtype=np.float32)
        in_maps.append(m)
    return in_maps


def run(inputs, trace=False):
    nc = _get_program()
    in_maps = make_in_maps(inputs)
    res = bass_utils.run_bass_kernel_spmd(nc, in_maps,
                                          core_ids=list(range(N_CORES)),
                                          trace=trace)
    outs = np.stack([res.results[b]["out"].T for b in range(N_CORES)])
    return outs.astype(np.float32), res


def kernel(**inputs):
    out, _ = run(inputs, trace=False)
    return out


# revision 15
# speedup vs baseline: 1.1629x; 1.0024x over previous
"""Bass/Trainium2 kernel for a 12-layer GPT-style transformer (nn_BERT).

Strategy: data-parallel over batch (B=8 -> 1 sequence per NeuronCore).
Each core runs all 12 layers on x^T [D=768, S=512] in "transposed"
activation layout (feature dim on partitions), f32r matmul datapath.

kernel(**inputs) takes the FULL unsharded inputs (as produced by
reference.setup_inputs()) and returns the full [8, 512, 768] output.
"""
import contextlib
import os
import sys
import types

sys.path.insert(0, "/opt/trn_rl_repo")
os.environ.setdefault("JAX_PLATFORMS", "axon")

import numpy as np

import concourse.bass as bass
import concourse.mybir as mybir
import concourse.tile as tile
from concourse import bacc
from concourse import bass_utils

F32 = mybir.dt.float32
F32R = mybir.dt.float32r
AF = mybir.ActivationFunctionType
OP = mybir.AluOpType

B, S, D, H, L, V = 8, 512, 768, 12, 12, 40478
DH = D // H            # 64
DF = 4 * D             # 3072
KC = D // 128          # 6 chunks of the model dim
KF = DF // 128         # 24 chunks of the ffn dim
SC = S // 128          # 4 chunks of the sequence
EPS = 1e-5

N_CORES = 8


def _install_ntff_hook():
    """Register the axon NTFF profiling hook that this image's antenv lacks."""
    if "antenv.axon_hooks" in sys.modules:
        return
    try:
        mod = types.ModuleType("antenv.axon_hooks")
        _h = [None]
        mod.set_axon_ntff_profile_hook = lambda h: _h.__setitem__(0, h)
        mod.get_axon_ntff_profile_hook = lambda: _h[0]
        sys.modules["antenv.axon_hooks"] = mod
        import antenv

        antenv.axon_hooks = mod
        if "/root/.axon_site" not in sys.path:
            sys.path.insert(0, "/root/.axon_site")
        from trn_agent_boot.trn_boot import _ntff_profile_via_ctypes

        mod.set_axon_ntff_profile_hook(
            _ntff_profile_via_ctypes("/opt/axon/libaxon_pjrt.so")
        )
    except Exception:
        pass


def build_program(n_layers=L, phases="ABCLD"):
    nc = bacc.Bacc("TRN2", target_bir_lowering=False, debug=False,
                   num_devices=N_CORES)

    d = {}
    d["x0"] = nc.dram_tensor("x0T", (D, S), F32R, kind="ExternalInput").ap()
    d["wqkv"] = nc.dram_tensor("wqkv", (n_layers, D, 3 * D), F32R, kind="ExternalInput").ap()
    d["bqkv"] = nc.dram_tensor("bqkv", (n_layers, 3 * D), F32, kind="ExternalInput").ap()
    d["wproj"] = nc.dram_tensor("wproj", (n_layers, D, D), F32R, kind="ExternalInput").ap()
    d["bproj"] = nc.dram_tensor("bproj", (n_layers, D), F32, kind="ExternalInput").ap()
    d["g1"] = nc.dram_tensor("g1", (n_layers, D), F32, kind="ExternalInput").ap()
    d["b1"] = nc.dram_tensor("b1", (n_layers, D), F32, kind="ExternalInput").ap()
    d["wfc"] = nc.dram_tensor("wfc", (n_layers, D, DF), F32R, kind="ExternalInput").ap()
    d["bfc"] = nc.dram_tensor("bfc", (n_layers, DF), F32, kind="ExternalInput").ap()
    d["wpr"] = nc.dram_tensor("wpr", (n_layers, DF, D), F32R, kind="ExternalInput").ap()
    d["bpr"] = nc.dram_tensor("bpr", (n_layers, D), F32, kind="ExternalInput").ap()
    d["g2"] = nc.dram_tensor("g2", (n_layers, D), F32, kind="ExternalInput").ap()
    d["b2"] = nc.dram_tensor("b2", (n_layers, D), F32, kind="ExternalInput").ap()
    d["triu"] = nc.dram_tensor("triu", (128, 128), F32R, kind="ExternalInput").ap()
    d["ones_row"] = nc.dram_tensor("ones_row", (1, 128), F32R, kind="ExternalInput").ap()
    d["ones_red"] = nc.dram_tensor("ones_red", (128, 2), F32R, kind="ExternalInput").ap()
    d["sel_den"] = nc.dram_tensor("sel_den", (128, 4, 4), F32R, kind="ExternalInput").ap()
    d["sel_bc2"] = nc.dram_tensor("sel_bc2", (4, 2, 128), F32R, kind="ExternalInput").ap()
    d["ones2d"] = nc.dram_tensor("ones2d", (128, 128), F32R, kind="ExternalInput").ap()
    d["out"] = nc.dram_tensor("out", (D, S), F32R, kind="ExternalOutput").ap()

    with tile.TileContext(nc) as tc, \
         nc.allow_low_precision(reason="f32r datapath; rel-err budget 2e-2"):
        _emit(tc, nc, n_layers, d, phases)
    nc.compile()
    return nc


def _emit(tc, nc, n_layers, d, phases="ABCLD"):
    ctx = contextlib.ExitStack()

    # --- long-lived SBUF pools -------------------------------------------
    consts = ctx.enter_context(tc.tile_pool(name="consts", bufs=1))
    # unions: tags share a slot across phases of a layer (qT->nT, kT->res1,
    # v->res2); aT and x get their own.
    uni = ctx.enter_context(tc.tile_pool(name="uni", bufs=1))
    x_pool = ctx.enter_context(tc.tile_pool(name="xp", bufs=2))
    probs_pool = ctx.enter_context(tc.tile_pool(name="probs", bufs=3))
    stats_pool = ctx.enter_context(tc.tile_pool(name="stats", bufs=2))
    stats1_pool = ctx.enter_context(tc.tile_pool(name="stats1", bufs=1))
    wq_pool = ctx.enter_context(tc.tile_pool(name="wq", bufs=2))
    wp_pool = ctx.enter_context(tc.tile_pool(name="wp", bufs=1))
    wf_pool = ctx.enter_context(tc.tile_pool(name="wf", bufs=2))
    wr_pool = ctx.enter_context(tc.tile_pool(name="wr", bufs=3))
    gelu_pool = ctx.enter_context(tc.tile_pool(name="gelu", bufs=3))
    bias_pool = ctx.enter_context(tc.tile_pool(name="bias", bufs=1))

    # constants
    cn = {}
    cn["triu"] = consts.tile([128, 128], F32R, tag="triu", name="triu")       # triu[p, f] = 1 if p <= f
    nc.sync.dma_start(out=cn["triu"], in_=d["triu"])
    cn["ones1"] = consts.tile([1, 128], F32R, tag="ones1", name="ones1")        # full bcast lhsT (K=1)
    nc.sync.dma_start(out=cn["ones1"], in_=d["ones_row"])
    cn["ones_red"] = consts.tile([128, 2], F32R, tag="ones_red", name="ones_red")     # partition-sum lhsT (M=2)
    nc.sync.dma_start(out=cn["ones_red"], in_=d["ones_red"])
    cn["sel_den"] = consts.tile([128, 4, 4], F32R, tag="sel_den", name="sel_den")
    nc.sync.dma_start(out=cn["sel_den"], in_=d["sel_den"])
    cn["sel_bc2"] = consts.tile([4, 2, 128], F32R, tag="sel_bc2", name="sel_bc2")
    nc.sync.dma_start(out=cn["sel_bc2"], in_=d["sel_bc2"])
    cn["ones2d"] = consts.tile([128, 128], F32R, tag="ones2d", name="ones2d")
    nc.sync.dma_start(out=cn["ones2d"], in_=d["ones2d"])
    cn["eps"] = consts.tile([1, 1], F32, tag="eps", name="eps")
    nc.vector.memset(cn["eps"], EPS)

    pools = dict(uni=uni, x=x_pool, probs=probs_pool, stats=stats_pool,
                 stats1=stats1_pool,
                 wq=wq_pool, wp=wp_pool, wf=wf_pool, wr=wr_pool,
                 gelu=gelu_pool, bias=bias_pool)

    # residual stream x^T, [128, KC, S] (chunk-major)
    x_cur = x_pool.tile([128, KC, S], F32R, tag="x")
    nc.sync.dma_start(out=x_cur, in_=d["x0"].rearrange("(k p) s -> p k s", p=128))

    for l in range(n_layers):
        with nc.named_scope(f"layer{l}"):
            x_cur = _layer(tc, nc, l, x_cur, d, cn, pools, phases)

    nc.sync.dma_start(out=d["out"].rearrange("(k p) s -> p k s", p=128), in_=x_cur)
    ctx.close()


def _ld_bias(nc, pool, dram_ap, tag, width):
    t = pool.tile([128, width], F32, tag=tag, name=tag)
    nc.sync.dma_start(out=t, in_=dram_ap.rearrange("(c p) -> p c", p=128))
    return t


def _layer(tc, nc, l, x_cur, d, cn, pools, phases="ABCLD"):
    uni = pools["uni"]; stats_pool = pools["stats"]; bias_pool = pools["bias"]
    stats1_pool = pools["stats1"]

    # ---- biases / ln params for this layer ------------------------------
    bqkv_t = _ld_bias(nc, bias_pool, d["bqkv"][l], "bqkv", 3 * D // 128)
    bv_b = bias_pool.tile([128, D], F32, tag="bvb")      # V bias, row-bcast
    nc.sync.dma_start(out=bv_b, in_=d["bqkv"][l, 2 * D:3 * D].partition_broadcast(128))
    bproj_t = _ld_bias(nc, bias_pool, d["bproj"][l], "bproj", KC)
    g1_t = _ld_bias(nc, bias_pool, d["g1"][l], "g1", KC)
    b1_t = _ld_bias(nc, bias_pool, d["b1"][l], "b1", KC)
    bfc_t = _ld_bias(nc, bias_pool, d["bfc"][l], "bfc", KF)
    bpr_t = _ld_bias(nc, bias_pool, d["bpr"][l], "bpr", KC)
    g2_t = _ld_bias(nc, bias_pool, d["g2"][l], "g2", KC)
    b2_t = _ld_bias(nc, bias_pool, d["b2"][l], "b2", KC)

    # =====================================================================
    # Phase A: qkv.  q^T,k^T transposed [128, KC, S]; V natural [sk, h, dh].
    # wqkv streamed in 6 column-parts of 384 (parts 0-3: Q,K; 4-5: V).
    # =====================================================================
    qT = uni.tile([128, KC, S], F32R, tag="u_q")         # -> nT later
    kT = uni.tile([128, KC, S], F32R, tag="u_k")         # -> res1 later
    v_nat = uni.tile([128, SC, H, DH], F32R, tag="u_v")  # -> res2 later

    with tc.tile_pool(name="ps_qk", bufs=3, space="PSUM") as ps_qk, \
         tc.tile_pool(name="ps_v", bufs=1, space="PSUM") as ps_v:
        for p in range(4):                 # Q/K parts: columns [384p, 384p+384)
            wpart = pools["wq"].tile([128, KC, 384], F32R, tag="wqkv")
            nc.sync.dma_start(
                out=wpart,
                in_=d["wqkv"][l, :, 384 * p:384 * (p + 1)].rearrange(
                    "(k q) n -> q k n", q=128))
            for j in range(3):
                oc = 3 * p + j             # output chunk of qk^T, 0..11
                pt = ps_qk.tile([128, S], F32)
                for k in range(KC):
                    nc.tensor.matmul(pt, wpart[:, k, 128 * j:128 * (j + 1)],
                                     x_cur[:, k, :], start=(k == 0),
                                     stop=(k == KC - 1))
                dst = qT[:, oc, :] if oc < KC else kT[:, oc - KC, :]
                nc.vector.tensor_scalar(out=dst, in0=pt,
                                        scalar1=bqkv_t[:, oc:oc + 1],
                                        scalar2=None, op0=OP.add)
        for p in range(4, 6):              # V parts: v-features [384(p-4), +384)
            wpart = pools["wq"].tile([128, KC, 384], F32R, tag="wqkv")
            nc.sync.dma_start(
                out=wpart,
                in_=d["wqkv"][l, :, 384 * p:384 * (p + 1)].rearrange(
                    "(k q) n -> q k n", q=128))
            n0 = 384 * (p - 4)
            for sc in range(SC):
                pv = ps_v.tile([128, 384], F32, tag="pv")
                for k in range(KC):
                    nc.tensor.matmul(pv, x_cur[:, k, 128 * sc:128 * (sc + 1)],
                                     wpart[:, k, :], start=(k == 0),
                                     stop=(k == KC - 1))
                # v_nat[:, sc, h, :] = pv + bias_v for heads n0/64 .. n0/64+5
                h0 = n0 // DH
                nc.vector.tensor_tensor(
                    out=v_nat[:, sc, h0:h0 + 6, :],
                    in0=pv.rearrange("q (h e) -> q h e", e=DH),
                    in1=bv_b[:, n0:n0 + 384].rearrange("q (h e) -> q h e", e=DH),
                    op=OP.add)

    if "B" not in phases:
        return qT
    # =====================================================================
    # Phase B: attention, head by head.  scores^T chunks via K^T.T @ Q^T.
    # =====================================================================
    aT = uni.tile([128, KC, S], F32R, tag="u_a")
    G = 4                                         # heads per recip group
    with tc.tile_pool(name="ps_sc", bufs=2, space="PSUM") as ps_sc, \
         tc.tile_pool(name="ps_av", bufs=1, space="PSUM") as ps_av, \
         tc.tile_pool(name="ps_dn", bufs=1, space="PSUM") as ps_dn, \
         tc.tile_pool(name="ps_bc", bufs=1, space="PSUM") as ps_bc:
        for g in range(H // G):
            pden = ps_dn.tile([G, S], F32, tag="den")
            pavs = []
            for j in range(G):
                h = G * g + j
                hc, hh = h // 2, (h % 2) * 64
                probs = pools["probs"].tile([128, SC, S], F32R, tag="probs")
                for c in range(SC):
                    n0 = 128 * c if c < SC - 1 else 256   # widened chunk 3
                    pt = ps_sc.tile([128, S], F32, tag="score")
                    nc.tensor.matmul(pt[:, 0:S - n0],
                                     kT[hh:hh + 64, hc, 128 * c:128 * c + 128],
                                     qT[hh:hh + 64, hc, n0:S],
                                     start=True, stop=True)
                    nc.scalar.activation(out=probs[:, c, n0:S],
                                         in_=pt[:, 0:S - n0],
                                         func=AF.Exp, scale=0.125)
                    if c == SC - 1:
                        # zero the fully-masked sq block [256, 384)
                        nc.vector.tensor_scalar(out=probs[:, c, 256:384],
                                                in0=probs[:, c, 256:384],
                                                scalar1=0.0, scalar2=None,
                                                op0=OP.mult)
                    nc.vector.tensor_tensor(
                        out=probs[:, c, 128 * c:128 * c + 128],
                        in0=probs[:, c, 128 * c:128 * c + 128],
                        in1=cn["triu"], op=OP.mult)
                # denominator rows, one-hot selected into pden row j
                for c in range(SC):
                    n0 = 128 * c if c < SC - 1 else 256
                    nc.tensor.matmul(pden[:, n0:S], cn["sel_den"][:, j, :],
                                     probs[:, c, n0:S],
                                     start=(j == 0 and c == 0),
                                     stop=(j == G - 1 and c == SC - 1),
                                     skip_group_check=True)
                # av^T accumulated over sk chunks; even head -> rows 0:64 of
                # its own psum, odd head -> rows 64:128 (pair-view lhsT).
                if hh == 0:
                    pav = ps_av.tile([64, S], F32, tag=f"av_e{j // 2}",
                                     name=f"pav_e{j // 2}")
                    lhsT = v_nat[:, :, h, :]
                else:
                    pav = ps_av.tile([128, S], F32, tag=f"av_o{j // 2}",
                                     name=f"pav_o{j // 2}")
                    lhsT = v_nat[:, :, h - 1:h + 1, :]
                pavs.append(pav)
                for c in range(SC):
                    n0 = 128 * c if c < SC - 1 else 256
                    if hh == 0:
                        lt = lhsT[:, c, :]
                        dst = pav[0:64, n0:S]
                    else:
                        lt = lhsT[:, c, :, :].rearrange("p h e -> p (h e)")
                        dst = pav[0:128, n0:S]
                    nc.tensor.matmul(dst, lt, probs[:, c, n0:S],
                                     start=(c == 0), stop=(c == SC - 1),
                                     skip_group_check=True)
            recip32 = pools["stats1"].tile([G, S], F32, tag="recip32")
            nc.vector.reciprocal_approx_fast(out=recip32, in_=pden[0:G, :])
            recip_r = pools["stats1"].tile([G, S], F32R, tag="recipr")
            nc.vector.tensor_copy(out=recip_r, in_=recip32)
            for q in range(2):                     # head pairs in this group
                hc = 2 * g + q
                pbc = ps_bc.tile([128, S], F32, tag="bc")
                nc.tensor.matmul(pbc, cn["sel_bc2"][:, q, :], recip_r,
                                 start=True, stop=True)
                bc_s = stats_pool.tile([128, S], F32, tag="bc_s")
                nc.vector.tensor_copy(out=bc_s, in_=pbc)
                nc.vector.tensor_tensor(out=aT[0:64, hc, :],
                                        in0=pavs[2 * q][0:64, :],
                                        in1=bc_s[0:64, :], op=OP.mult)
                nc.vector.tensor_tensor(out=aT[64:128, hc, :],
                                        in0=pavs[2 * q + 1][64:128, :],
                                        in1=bc_s[64:128, :], op=OP.mult)

    # =====================================================================
    # Phase C: attn out proj + residual + LN1
    # =====================================================================
    wproj_t = pools["wp"].tile([128, KC, D], F32R, tag="wproj")
    nc.sync.dma_start(out=wproj_t,
                      in_=d["wproj"][l].rearrange("(k p) n -> p k n", p=128))
    res1 = uni.tile([128, KC, S], F32R, tag="u_k")       # reuses kT slot
    with tc.tile_pool(name="ps_pj", bufs=3, space="PSUM") as ps_pj:
        for oc in range(KC):
            pt = ps_pj.tile([128, S], F32)
            for k in range(KC):
                nc.tensor.matmul(pt, wproj_t[:, k, 128 * oc:128 * (oc + 1)],
                                 aT[:, k, :], start=(k == 0), stop=(k == KC - 1))
            nc.vector.tensor_scalar(out=pt, in0=pt, scalar1=bproj_t[:, oc:oc + 1],
                                    scalar2=None, op0=OP.add)
            nc.vector.tensor_tensor(out=res1[:, oc, :], in0=pt,
                                    in1=x_cur[:, oc, :], op=OP.add)

    if "L" not in phases:
        return res1
    nT = uni.tile([128, KC, S], F32R, tag="u_q")         # reuses qT slot
    _layernorm(tc, nc, res1, nT, g1_t, b1_t, cn, stats_pool, stats1_pool, "ln1")

    if "D" not in phases:
        return nT
    # =====================================================================
    # Phase D: fused fc -> gelu -> pr (+ residual), k-outer over DF chunks.
    # wfc streamed in 6 column-parts of 512 (4 kf each); wpr in row-slices.
    # =====================================================================
    res2 = uni.tile([128, KC, S], F32R, tag="u_v")       # reuses v_nat slot
    with tc.tile_pool(name="ps_pr", bufs=1, space="PSUM") as ps_pr, \
         tc.tile_pool(name="ps_fc", bufs=2, space="PSUM") as ps_fc:
        pr_acc = [ps_pr.tile([128, S], F32, tag=f"pr{oc}", name=f"pr{oc}") for oc in range(KC)]
        for part in range(6):
            wfc_p = pools["wf"].tile([128, KC, 512], F32R, tag="wfc")
            nc.sync.dma_start(
                out=wfc_p,
                in_=d["wfc"][l, :, 512 * part:512 * (part + 1)].rearrange(
                    "(k q) n -> q k n", q=128))
            for j in range(4):
                kf = 4 * part + j
                wpr_k = pools["wr"].tile([128, D], F32R, tag="wprk")
                nc.sync.dma_start(out=wpr_k,
                                  in_=d["wpr"][l, 128 * kf:128 * (kf + 1), :])
                pf = ps_fc.tile([128, S], F32)
                for k in range(KC):
                    nc.tensor.matmul(pf, wfc_p[:, k, 128 * j:128 * (j + 1)],
                                     nT[:, k, :], start=(k == 0),
                                     stop=(k == KC - 1))
                gk = pools["gelu"].tile([128, S], F32R, tag="gk")
                nc.scalar.activation(out=gk, in_=pf, func=AF.Gelu_apprx_tanh,
                                     bias=bfc_t[:, kf:kf + 1], scale=1.0)
                for oc in range(KC):
                    nc.tensor.matmul(pr_acc[oc],
                                     wpr_k[:, 128 * oc:128 * (oc + 1)],
                                     gk, start=(kf == 0), stop=(kf == KF - 1))
        for oc in range(KC):
            nc.vector.tensor_scalar(out=pr_acc[oc], in0=pr_acc[oc],
                                    scalar1=bpr_t[:, oc:oc + 1],
                                    scalar2=None, op0=OP.add)
            nc.vector.tensor_tensor(out=res2[:, oc, :], in0=pr_acc[oc],
                                    in1=nT[:, oc, :], op=OP.add)

    x_next = pools["x"].tile([128, KC, S], F32R, tag="x")
    _layernorm(tc, nc, res2, x_next, g2_t, b2_t, cn, stats_pool, stats1_pool, "ln2")
    return x_next


def _layernorm(tc, nc, src, dst, g_t, b_t, cn, stats_pool, stats1_pool, tag):
    """LN over the partition (feature) axis of src [128, KC, S] -> dst."""
    with tc.tile_pool(name=f"ps_{tag}", bufs=1, space="PSUM") as ps:
        psums = ps.tile([128, S], F32, tag="bsum")   # every row = sum(x)
        psq = ps.tile([2, S], F32, tag="s1")         # row 0: sum(x^2)
        for k in range(KC):
            sq = stats_pool.tile([128, S], F32R, tag="lnsq")
            nc.scalar.activation(out=sq, in_=src[:, k, :], func=AF.Square)
            nc.tensor.matmul(psums, cn["ones2d"], src[:, k, :],
                             start=(k == 0), stop=(k == KC - 1))
            nc.tensor.matmul(psq, cn["ones_red"], sq,
                             start=(k == 0), stop=(k == KC - 1))
        # move broadcasts to SBUF promptly so the PSUM banks free early
        bsum_s = stats1_pool.tile([128, S], F32, tag="bsum_s")
        nc.vector.tensor_copy(out=bsum_s, in_=psums)
        mu1 = stats1_pool.tile([1, S], F32, tag="mu1")
        var = stats1_pool.tile([1, S], F32, tag="var")
        rsd = stats1_pool.tile([1, S], F32R, tag="rsd")
        nc.vector.tensor_scalar(out=mu1, in0=bsum_s[0:1, :], scalar1=1.0 / D,
                                scalar2=None, op0=OP.mult)
        nc.vector.tensor_tensor(out=var, in0=mu1, in1=mu1, op=OP.mult)
        nc.vector.scalar_tensor_tensor(out=var, in0=psq[0:1, :], scalar=1.0 / D,
                                       in1=var, op0=OP.mult, op1=OP.subtract)
        nc.scalar.activation(out=var, in_=var, func=AF.Sqrt, bias=cn["eps"])
        rsd32 = stats1_pool.tile([1, S], F32, tag="rsd32")
        nc.vector.reciprocal_approx_fast(out=rsd32, in_=var)
        nc.vector.tensor_copy(out=rsd, in_=rsd32)
        prs = ps.tile([128, S], F32, tag="s1")       # reuse the psq bank
        nc.tensor.matmul(prs, cn["ones1"], rsd, start=True, stop=True)
        brs_s = stats1_pool.tile([128, S], F32, tag="brs_s")
        nc.vector.tensor_copy(out=brs_s, in_=prs)
        for k in range(KC):
            t = stats_pool.tile([128, S], F32, tag="lnt")
            # t = src - mean  (mean folded from the broadcast sums)
            nc.vector.scalar_tensor_tensor(out=t, in0=bsum_s, scalar=-1.0 / D,
                                           in1=src[:, k, :], op0=OP.mult,
                                           op1=OP.add)
            nc.vector.tensor_tensor(out=t, in0=t, in1=brs_s, op=OP.mult)
            nc.vector.tensor_scalar(out=dst[:, k, :], in0=t,
                                    scalar1=g_t[:, k:k + 1],
                                    scalar2=b_t[:, k:k + 1],
                                    op0=OP.mult, op1=OP.add)



# =========================================================================
# Host side
# =========================================================================
_CACHE = {}


def _get_program():
    if "nc" not in _CACHE:
        _install_ntff_hook()
        _CACHE["nc"] = build_program(L)
    return _CACHE["nc"]


def make_in_maps(inputs, n_layers=L):
    tokens = np.asarray(inputs["tokens"])
    we = np.asarray(inputs["we"], dtype=np.float32)
    pos = we[V:V + S]                                  # [S, D]
    triu = np.triu(np.ones((128, 128), dtype=np.float32))

    def f32(name):
        return np.ascontiguousarray(np.asarray(inputs[name])[:n_layers],
                                    dtype=np.float32)

    shared = {k: f32(k) for k in ["wqkv", "bqkv", "wproj", "bproj", "g1", "b1",
                                  "wfc", "bfc", "wpr", "bpr", "g2", "b2"]}
    shared["triu"] = triu
    shared["ones_row"] = np.ones((1, 128), dtype=np.float32)
    onesred = np.zeros((128, 2), dtype=np.float32); onesred[:, 0] = 1.0
    shared["ones_red"] = onesred
    sel_den = np.zeros((128, 4, 4), dtype=np.float32)
    for j in range(4):
        sel_den[:, j, j] = 1.0
    shared["sel_den"] = sel_den
    sel_bc2 = np.zeros((4, 2, 128), dtype=np.float32)
    for q in range(2):
        sel_bc2[2 * q, q, 0:64] = 1.0
        sel_bc2[2 * q + 1, q, 64:128] = 1.0
    shared["sel_bc2"] = sel_bc2
    shared["ones2d"] = np.ones((128, 128), dtype=np.float32)
    in_maps = []
    for b in range(N_CORES):
        x0 = we[tokens[b]] + pos                       # [S, D]
        m = dict(shared)
        m["x0T"] = np.ascontiguousarray(x0.T, dtype=np.float32)
        in_maps.append(m)
    return in_maps


def run(inputs, trace=False):
    nc = _get_program()
    in_maps = make_in_maps(inputs)
    res = bass_utils.run_bass_kernel_spmd(nc, in_maps,
                                          core_ids=list(range(N_CORES)),
                                          trace=trace)
    outs = np.stack([res.results[b]["out"].T for b in range(N_CORES)])
    return outs.astype(np.float32), res


def kernel(**inputs):
    out, _ = run(inputs, trace=False)
    return out


# revision 16
# speedup vs baseline: 1.1962x; 1.0287x over previous
"""Bass/Trainium2 kernel for a 12-layer GPT-style transformer (nn_BERT).

Strategy: data-parallel over batch (B=8 -> 1 sequence per NeuronCore).
Each core runs all 12 layers on x^T [D=768, S=512] in "transposed"
activation layout (feature dim on partitions), f32r matmul datapath.

kernel(**inputs) takes the FULL unsharded inputs (as produced by
reference.setup_inputs()) and returns the full [8, 512, 768] output.
"""
import contextlib
import os
import sys
import types

sys.path.insert(0, "/opt/trn_rl_repo")
os.environ.setdefault("JAX_PLATFORMS", "axon")

import numpy as np

import concourse.bass as bass
import concourse.mybir as mybir
import concourse.tile as tile
from concourse import bacc
from concourse import bass_utils

F32 = mybir.dt.float32
F32R = mybir.dt.float32r
AF = mybir.ActivationFunctionType
OP = mybir.AluOpType

B, S, D, H, L, V = 8, 512, 768, 12, 12, 40478
DH = D // H            # 64
DF = 4 * D             # 3072
KC = D // 128          # 6 chunks of the model dim
KF = DF // 128         # 24 chunks of the ffn dim
SC = S // 128          # 4 chunks of the sequence
EPS = 1e-5

N_CORES = 8


def _install_ntff_hook():
    """Register the axon NTFF profiling hook that this image's antenv lacks."""
    if "antenv.axon_hooks" in sys.modules:
        return
    try:
        mod = types.ModuleType("antenv.axon_hooks")
        _h = [None]
        mod.set_axon_ntff_profile_hook = lambda h: _h.__setitem__(0, h)
        mod.get_axon_ntff_profile_hook = lambda: _h[0]
        sys.modules["antenv.axon_hooks"] = mod
        import antenv

        antenv.axon_hooks = mod
        if "/root/.axon_site" not in sys.path:
            sys.path.insert(0, "/root/.axon_site")
        from trn_agent_boot.trn_boot import _ntff_profile_via_ctypes

        mod.set_axon_ntff_profile_hook(
            _ntff_profile_via_ctypes("/opt/axon/libaxon_pjrt.so")
        )
    except Exception:
        pass


def build_program(n_layers=L, phases="ABCLD"):
    nc = bacc.Bacc("TRN2", target_bir_lowering=False, debug=False,
                   num_devices=N_CORES)

    d = {}
    d["x0"] = nc.dram_tensor("x0T", (D, S), F32R, kind="ExternalInput").ap()
    d["wqkv"] = nc.dram_tensor("wqkv", (n_layers, D, 3 * D), F32R, kind="ExternalInput").ap()
    d["bqkv"] = nc.dram_tensor("bqkv", (n_layers, 3 * D), F32, kind="ExternalInput").ap()
    d["wproj"] = nc.dram_tensor("wproj", (n_layers, D, D), F32R, kind="ExternalInput").ap()
    d["bproj"] = nc.dram_tensor("bproj", (n_layers, D), F32, kind="ExternalInput").ap()
    d["g1"] = nc.dram_tensor("g1", (n_layers, D), F32, kind="ExternalInput").ap()
    d["b1"] = nc.dram_tensor("b1", (n_layers, D), F32, kind="ExternalInput").ap()
    d["wfc"] = nc.dram_tensor("wfc", (n_layers, D, DF), F32R, kind="ExternalInput").ap()
    d["bfc"] = nc.dram_tensor("bfc", (n_layers, DF), F32, kind="ExternalInput").ap()
    d["wpr"] = nc.dram_tensor("wpr", (n_layers, DF, D), F32R, kind="ExternalInput").ap()
    d["bpr"] = nc.dram_tensor("bpr", (n_layers, D), F32, kind="ExternalInput").ap()
    d["g2"] = nc.dram_tensor("g2", (n_layers, D), F32, kind="ExternalInput").ap()
    d["b2"] = nc.dram_tensor("b2", (n_layers, D), F32, kind="ExternalInput").ap()
    d["triu"] = nc.dram_tensor("triu", (128, 128), F32R, kind="ExternalInput").ap()
    d["ones_row"] = nc.dram_tensor("ones_row", (1, 128), F32R, kind="ExternalInput").ap()
    d["ones_red"] = nc.dram_tensor("ones_red", (128, 2), F32R, kind="ExternalInput").ap()
    d["sel_den"] = nc.dram_tensor("sel_den", (128, 4, 4), F32R, kind="ExternalInput").ap()
    d["sel_bc2"] = nc.dram_tensor("sel_bc2", (4, 2, 128), F32R, kind="ExternalInput").ap()
    d["ones2d"] = nc.dram_tensor("ones2d", (128, 128), F32R, kind="ExternalInput").ap()
    d["ident"] = nc.dram_tensor("ident", (128, 128), F32R, kind="ExternalInput").ap()
    d["out"] = nc.dram_tensor("out", (D, S), F32R, kind="ExternalOutput").ap()

    with tile.TileContext(nc) as tc, \
         nc.allow_low_precision(reason="f32r datapath; rel-err budget 2e-2"):
        _emit(tc, nc, n_layers, d, phases)
    nc.compile()
    return nc


def _emit(tc, nc, n_layers, d, phases="ABCLD"):
    ctx = contextlib.ExitStack()

    # --- long-lived SBUF pools -------------------------------------------
    consts = ctx.enter_context(tc.tile_pool(name="consts", bufs=1))
    # unions: tags share a slot across phases of a layer (qT->nT, kT->res1,
    # v->res2); aT and x get their own.
    uni = ctx.enter_context(tc.tile_pool(name="uni", bufs=1))
    x_pool = ctx.enter_context(tc.tile_pool(name="xp", bufs=2))
    probs_pool = ctx.enter_context(tc.tile_pool(name="probs", bufs=3))
    stats_pool = ctx.enter_context(tc.tile_pool(name="stats", bufs=2))
    stats1_pool = ctx.enter_context(tc.tile_pool(name="stats1", bufs=1))
    wq_pool = ctx.enter_context(tc.tile_pool(name="wq", bufs=2))
    wp_pool = ctx.enter_context(tc.tile_pool(name="wp", bufs=1))
    wf_pool = ctx.enter_context(tc.tile_pool(name="wf", bufs=2))
    wr_pool = ctx.enter_context(tc.tile_pool(name="wr", bufs=3))
    gelu_pool = ctx.enter_context(tc.tile_pool(name="gelu", bufs=3))
    bias_pool = ctx.enter_context(tc.tile_pool(name="bias", bufs=1))

    # constants
    cn = {}
    cn["triu"] = consts.tile([128, 128], F32R, tag="triu", name="triu")       # triu[p, f] = 1 if p <= f
    nc.sync.dma_start(out=cn["triu"], in_=d["triu"])
    cn["ones1"] = consts.tile([1, 128], F32R, tag="ones1", name="ones1")        # full bcast lhsT (K=1)
    nc.sync.dma_start(out=cn["ones1"], in_=d["ones_row"])
    cn["ones_red"] = consts.tile([128, 2], F32R, tag="ones_red", name="ones_red")     # partition-sum lhsT (M=2)
    nc.sync.dma_start(out=cn["ones_red"], in_=d["ones_red"])
    cn["sel_den"] = consts.tile([128, 4, 4], F32R, tag="sel_den", name="sel_den")
    nc.sync.dma_start(out=cn["sel_den"], in_=d["sel_den"])
    cn["sel_bc2"] = consts.tile([4, 2, 128], F32R, tag="sel_bc2", name="sel_bc2")
    nc.sync.dma_start(out=cn["sel_bc2"], in_=d["sel_bc2"])
    cn["ones2d"] = consts.tile([128, 128], F32R, tag="ones2d", name="ones2d")
    nc.sync.dma_start(out=cn["ones2d"], in_=d["ones2d"])
    cn["ident"] = consts.tile([128, 128], F32R, tag="ident", name="ident")
    nc.sync.dma_start(out=cn["ident"], in_=d["ident"])
    cn["eps"] = consts.tile([1, 1], F32, tag="eps", name="eps")
    nc.vector.memset(cn["eps"], EPS)

    pools = dict(uni=uni, x=x_pool, probs=probs_pool, stats=stats_pool,
                 stats1=stats1_pool,
                 wq=wq_pool, wp=wp_pool, wf=wf_pool, wr=wr_pool,
                 gelu=gelu_pool, bias=bias_pool)

    # residual stream x^T, [128, KC, S] (chunk-major)
    x_cur = x_pool.tile([128, KC, S], F32R, tag="x")
    nc.sync.dma_start(out=x_cur, in_=d["x0"].rearrange("(k p) s -> p k s", p=128))

    for l in range(n_layers):
        with nc.named_scope(f"layer{l}"):
            x_cur = _layer(tc, nc, l, x_cur, d, cn, pools, phases)

    nc.sync.dma_start(out=d["out"].rearrange("(k p) s -> p k s", p=128), in_=x_cur)
    ctx.close()


def _ld_bias(nc, pool, dram_ap, tag, width):
    t = pool.tile([128, width], F32, tag=tag, name=tag)
    nc.sync.dma_start(out=t, in_=dram_ap.rearrange("(c p) -> p c", p=128))
    return t


def _layer(tc, nc, l, x_cur, d, cn, pools, phases="ABCLD"):
    uni = pools["uni"]; stats_pool = pools["stats"]; bias_pool = pools["bias"]
    stats1_pool = pools["stats1"]

    # ---- biases / ln params for this layer ------------------------------
    bqkv_t = _ld_bias(nc, bias_pool, d["bqkv"][l], "bqkv", 3 * D // 128)
    bv_b = bias_pool.tile([128, D], F32, tag="bvb")      # V bias, row-bcast
    nc.sync.dma_start(out=bv_b, in_=d["bqkv"][l, 2 * D:3 * D].partition_broadcast(128))
    bproj_t = _ld_bias(nc, bias_pool, d["bproj"][l], "bproj", KC)
    g1_t = _ld_bias(nc, bias_pool, d["g1"][l], "g1", KC)
    b1_t = _ld_bias(nc, bias_pool, d["b1"][l], "b1", KC)
    bfc_t = _ld_bias(nc, bias_pool, d["bfc"][l], "bfc", KF)
    bpr_t = _ld_bias(nc, bias_pool, d["bpr"][l], "bpr", KC)
    g2_t = _ld_bias(nc, bias_pool, d["g2"][l], "g2", KC)
    b2_t = _ld_bias(nc, bias_pool, d["b2"][l], "b2", KC)

    # =====================================================================
    # Phase A: qkv.  q^T,k^T transposed [128, KC, S]; V natural [sk, h, dh].
    # wqkv streamed in 6 column-parts of 384 (parts 0-3: Q,K; 4-5: V).
    # =====================================================================
    qT = uni.tile([128, KC, S], F32R, tag="u_q")         # -> nT later
    kT = uni.tile([128, KC, S], F32R, tag="u_k")         # -> res1 later
    v_nat = uni.tile([128, SC, H, DH], F32R, tag="u_v")  # -> res2 later

    with tc.tile_pool(name="ps_qk", bufs=3, space="PSUM") as ps_qk, \
         tc.tile_pool(name="ps_v", bufs=1, space="PSUM") as ps_v:
        for p in range(4):                 # Q/K parts: columns [384p, 384p+384)
            wpart = pools["wq"].tile([128, KC, 384], F32R, tag="wqkv")
            nc.sync.dma_start(
                out=wpart,
                in_=d["wqkv"][l, :, 384 * p:384 * (p + 1)].rearrange(
                    "(k q) n -> q k n", q=128))
            for j in range(3):
                oc = 3 * p + j             # output chunk of qk^T, 0..11
                pt = ps_qk.tile([128, S], F32)
                for k in range(KC):
                    nc.tensor.matmul(pt, wpart[:, k, 128 * j:128 * (j + 1)],
                                     x_cur[:, k, :], start=(k == 0),
                                     stop=(k == KC - 1))
                dst = qT[:, oc, :] if oc < KC else kT[:, oc - KC, :]
                nc.vector.tensor_scalar(out=dst, in0=pt,
                                        scalar1=bqkv_t[:, oc:oc + 1],
                                        scalar2=None, op0=OP.add)
        for p in range(4, 6):              # V parts: v-features [384(p-4), +384)
            wpart = pools["wq"].tile([128, KC, 384], F32R, tag="wqkv")
            nc.sync.dma_start(
                out=wpart,
                in_=d["wqkv"][l, :, 384 * p:384 * (p + 1)].rearrange(
                    "(k q) n -> q k n", q=128))
            n0 = 384 * (p - 4)
            for sc in range(SC):
                pv = ps_v.tile([128, 384], F32, tag="pv")
                for k in range(KC):
                    nc.tensor.matmul(pv, x_cur[:, k, 128 * sc:128 * (sc + 1)],
                                     wpart[:, k, :], start=(k == 0),
                                     stop=(k == KC - 1))
                # v_nat[:, sc, h, :] = pv + bias_v for heads n0/64 .. n0/64+5
                h0 = n0 // DH
                nc.vector.tensor_tensor(
                    out=v_nat[:, sc, h0:h0 + 6, :],
                    in0=pv.rearrange("q (h e) -> q h e", e=DH),
                    in1=bv_b[:, n0:n0 + 384].rearrange("q (h e) -> q h e", e=DH),
                    op=OP.add)

    if "B" not in phases:
        return qT
    # =====================================================================
    # Phase B: attention, head by head.  scores^T chunks via K^T.T @ Q^T.
    # =====================================================================
    aT = uni.tile([128, KC, S], F32R, tag="u_a")
    G = 4                                         # heads per recip group
    with tc.tile_pool(name="ps_sc", bufs=2, space="PSUM") as ps_sc, \
         tc.tile_pool(name="ps_av", bufs=1, space="PSUM") as ps_av, \
         tc.tile_pool(name="ps_dn", bufs=1, space="PSUM") as ps_dn, \
         tc.tile_pool(name="ps_bc", bufs=1, space="PSUM") as ps_bc:
        for g in range(H // G):
            pden = ps_dn.tile([G, S], F32, tag="den")
            pavs = []
            for j in range(G):
                h = G * g + j
                hc, hh = h // 2, (h % 2) * 64
                probs = pools["probs"].tile([128, SC, S], F32R, tag="probs")
                for c in range(SC):
                    n0 = 128 * c if c < SC - 1 else 256   # widened chunk 3
                    pt = ps_sc.tile([128, S], F32, tag="score")
                    nc.tensor.matmul(pt[:, 0:S - n0],
                                     kT[hh:hh + 64, hc, 128 * c:128 * c + 128],
                                     qT[hh:hh + 64, hc, n0:S],
                                     start=True, stop=True)
                    nc.scalar.activation(out=probs[:, c, n0:S],
                                         in_=pt[:, 0:S - n0],
                                         func=AF.Exp, scale=0.125)
                    if c == SC - 1:
                        # zero the fully-masked sq block [256, 384)
                        nc.vector.tensor_scalar(out=probs[:, c, 256:384],
                                                in0=probs[:, c, 256:384],
                                                scalar1=0.0, scalar2=None,
                                                op0=OP.mult)
                    nc.vector.tensor_tensor(
                        out=probs[:, c, 128 * c:128 * c + 128],
                        in0=probs[:, c, 128 * c:128 * c + 128],
                        in1=cn["triu"], op=OP.mult)
                # denominator rows, one-hot selected into pden row j
                for c in range(SC):
                    n0 = 128 * c if c < SC - 1 else 256
                    nc.tensor.matmul(pden[:, n0:S], cn["sel_den"][:, j, :],
                                     probs[:, c, n0:S],
                                     start=(j == 0 and c == 0),
                                     stop=(j == G - 1 and c == SC - 1),
                                     skip_group_check=True)
                # av^T accumulated over sk chunks; even head -> rows 0:64 of
                # its own psum, odd head -> rows 64:128 (pair-view lhsT).
                if hh == 0:
                    pav = ps_av.tile([64, S], F32, tag=f"av_e{j // 2}",
                                     name=f"pav_e{j // 2}")
                    lhsT = v_nat[:, :, h, :]
                else:
                    pav = ps_av.tile([128, S], F32, tag=f"av_o{j // 2}",
                                     name=f"pav_o{j // 2}")
                    lhsT = v_nat[:, :, h - 1:h + 1, :]
                pavs.append(pav)
                for c in range(SC):
                    n0 = 128 * c if c < SC - 1 else 256
                    if hh == 0:
                        lt = lhsT[:, c, :]
                        dst = pav[0:64, n0:S]
                    else:
                        lt = lhsT[:, c, :, :].rearrange("p h e -> p (h e)")
                        dst = pav[0:128, n0:S]
                    nc.tensor.matmul(dst, lt, probs[:, c, n0:S],
                                     start=(c == 0), stop=(c == SC - 1),
                                     skip_group_check=True)
            recip32 = pools["stats1"].tile([G, S], F32, tag="recip32")
            nc.vector.reciprocal_approx_fast(out=recip32, in_=pden[0:G, :])
            recip_r = pools["stats1"].tile([G, S], F32R, tag="recipr")
            nc.vector.tensor_copy(out=recip_r, in_=recip32)
            for q in range(2):                     # head pairs in this group
                hc = 2 * g + q
                pbc = ps_bc.tile([128, S], F32, tag="bc")
                nc.tensor.matmul(pbc, cn["sel_bc2"][:, q, :], recip_r,
                                 start=True, stop=True)
                bc_s = stats_pool.tile([128, S], F32, tag="bc_s")
                nc.vector.tensor_copy(out=bc_s, in_=pbc)
                nc.vector.tensor_tensor(out=aT[0:64, hc, :],
                                        in0=pavs[2 * q][0:64, :],
                                        in1=bc_s[0:64, :], op=OP.mult)
                nc.vector.tensor_tensor(out=aT[64:128, hc, :],
                                        in0=pavs[2 * q + 1][64:128, :],
                                        in1=bc_s[64:128, :], op=OP.mult)

    # =====================================================================
    # Phase C: attn out proj + residual + LN1
    # =====================================================================
    wproj_t = pools["wp"].tile([128, KC, D], F32R, tag="wproj")
    nc.sync.dma_start(out=wproj_t,
                      in_=d["wproj"][l].rearrange("(k p) n -> p k n", p=128))
    res1 = uni.tile([128, KC, S], F32R, tag="u_k")       # reuses kT slot
    with tc.tile_pool(name="ps_pj", bufs=3, space="PSUM") as ps_pj:
        for oc in range(KC):
            pt = ps_pj.tile([128, S], F32)
            for k in range(KC):
                nc.tensor.matmul(pt, wproj_t[:, k, 128 * oc:128 * (oc + 1)],
                                 aT[:, k, :], start=(k == 0), stop=False)
            nc.tensor.matmul(pt, cn["ident"], x_cur[:, oc, :],
                             start=False, stop=True)
            nc.scalar.activation(out=res1[:, oc, :], in_=pt, func=AF.Identity,
                                 bias=bproj_t[:, oc:oc + 1], scale=1.0)

    if "L" not in phases:
        return res1
    nT = uni.tile([128, KC, S], F32R, tag="u_q")         # reuses qT slot
    _layernorm(tc, nc, res1, nT, g1_t, b1_t, cn, stats_pool, stats1_pool, "ln1")

    if "D" not in phases:
        return nT
    # =====================================================================
    # Phase D: fused fc -> gelu -> pr (+ residual), k-outer over DF chunks.
    # wfc streamed in 6 column-parts of 512 (4 kf each); wpr in row-slices.
    # =====================================================================
    res2 = uni.tile([128, KC, S], F32R, tag="u_v")       # reuses v_nat slot
    with tc.tile_pool(name="ps_pr", bufs=1, space="PSUM") as ps_pr, \
         tc.tile_pool(name="ps_fc", bufs=2, space="PSUM") as ps_fc:
        pr_acc = [ps_pr.tile([128, S], F32, tag=f"pr{oc}", name=f"pr{oc}") for oc in range(KC)]
        for part in range(6):
            wfc_p = pools["wf"].tile([128, KC, 512], F32R, tag="wfc")
            nc.sync.dma_start(
                out=wfc_p,
                in_=d["wfc"][l, :, 512 * part:512 * (part + 1)].rearrange(
                    "(k q) n -> q k n", q=128))
            for j in range(4):
                kf = 4 * part + j
                wpr_k = pools["wr"].tile([128, D], F32R, tag="wprk")
                nc.sync.dma_start(out=wpr_k,
                                  in_=d["wpr"][l, 128 * kf:128 * (kf + 1), :])
                pf = ps_fc.tile([128, S], F32)
                for k in range(KC):
                    nc.tensor.matmul(pf, wfc_p[:, k, 128 * j:128 * (j + 1)],
                                     nT[:, k, :], start=(k == 0),
                                     stop=(k == KC - 1))
                gk = pools["gelu"].tile([128, S], F32R, tag="gk")
                nc.scalar.activation(out=gk, in_=pf, func=AF.Gelu_apprx_tanh,
                                     bias=bfc_t[:, kf:kf + 1], scale=1.0)
                for oc in range(KC):
                    nc.tensor.matmul(pr_acc[oc],
                                     wpr_k[:, 128 * oc:128 * (oc + 1)],
                                     gk, start=(kf == 0), stop=False)
        for oc in range(KC):
            nc.tensor.matmul(pr_acc[oc], cn["ident"], nT[:, oc, :],
                             start=False, stop=True)
            nc.scalar.activation(out=res2[:, oc, :], in_=pr_acc[oc],
                                 func=AF.Identity,
                                 bias=bpr_t[:, oc:oc + 1], scale=1.0)

    x_next = pools["x"].tile([128, KC, S], F32R, tag="x")
    _layernorm(tc, nc, res2, x_next, g2_t, b2_t, cn, stats_pool, stats1_pool, "ln2")
    return x_next


def _layernorm(tc, nc, src, dst, g_t, b_t, cn, stats_pool, stats1_pool, tag):
    """LN over the partition (feature) axis of src [128, KC, S] -> dst."""
    with tc.tile_pool(name=f"ps_{tag}", bufs=1, space="PSUM") as ps:
        psums = ps.tile([128, S], F32, tag="bsum")   # every row = sum(x)
        psq = ps.tile([2, S], F32, tag="s1")         # row 0: sum(x^2)
        for k in range(KC):
            sq = stats_pool.tile([128, S], F32R, tag="lnsq")
            nc.scalar.activation(out=sq, in_=src[:, k, :], func=AF.Square)
            nc.tensor.matmul(psums, cn["ones2d"], src[:, k, :],
                             start=(k == 0), stop=(k == KC - 1))
            nc.tensor.matmul(psq, cn["ones_red"], sq,
                             start=(k == 0), stop=(k == KC - 1))
        # move broadcasts to SBUF promptly so the PSUM banks free early
        bsum_s = stats1_pool.tile([128, S], F32, tag="bsum_s")
        nc.vector.tensor_copy(out=bsum_s, in_=psums)
        mu1 = stats1_pool.tile([1, S], F32, tag="mu1")
        var = stats1_pool.tile([1, S], F32, tag="var")
        rsd = stats1_pool.tile([1, S], F32R, tag="rsd")
        nc.vector.tensor_scalar(out=mu1, in0=bsum_s[0:1, :], scalar1=1.0 / D,
                                scalar2=None, op0=OP.mult)
        nc.vector.tensor_tensor(out=var, in0=mu1, in1=mu1, op=OP.mult)
        nc.vector.scalar_tensor_tensor(out=var, in0=psq[0:1, :], scalar=1.0 / D,
                                       in1=var, op0=OP.mult, op1=OP.subtract)
        nc.scalar.activation(out=var, in_=var, func=AF.Sqrt, bias=cn["eps"])
        rsd32 = stats1_pool.tile([1, S], F32, tag="rsd32")
        nc.vector.reciprocal_approx_fast(out=rsd32, in_=var)
        nc.vector.tensor_copy(out=rsd, in_=rsd32)
        prs = ps.tile([128, S], F32, tag="s1")       # reuse the psq bank
        nc.tensor.matmul(prs, cn["ones1"], rsd, start=True, stop=True)
        brs_s = stats1_pool.tile([128, S], F32, tag="brs_s")
        nc.vector.tensor_copy(out=brs_s, in_=prs)
        for k in range(KC):
            t = stats_pool.tile([128, S], F32, tag="lnt")
            # t = src - mean  (mean folded from the broadcast sums)
            nc.vector.scalar_tensor_tensor(out=t, in0=bsum_s, scalar=-1.0 / D,
                                           in1=src[:, k, :], op0=OP.mult,
                                           op1=OP.add)
            nc.vector.tensor_tensor(out=t, in0=t, in1=brs_s, op=OP.mult)
            nc.scalar.activation(out=dst[:, k, :], in_=t, func=AF.Identity,
                                 bias=b_t[:, k:k + 1],
                                 scale=g_t[:, k:k + 1])



# =========================================================================
# Host side
# =========================================================================
_CACHE = {}


def _get_program():
    if "nc" not in _CACHE:
        _install_ntff_hook()
        _CACHE["nc"] = build_program(L)
    return _CACHE["nc"]


def make_in_maps(inputs, n_layers=L):
    tokens = np.asarray(inputs["tokens"])
    we = np.asarray(inputs["we"], dtype=np.float32)
    pos = we[V:V + S]                                  # [S, D]
    triu = np.triu(np.ones((128, 128), dtype=np.float32))

    def f32(name):
        return np.ascontiguousarray(np.asarray(inputs[name])[:n_layers],
                                    dtype=np.float32)

    shared = {k: f32(k) for k in ["wqkv", "bqkv", "wproj", "bproj", "g1", "b1",
                                  "wfc", "bfc", "wpr", "bpr", "g2", "b2"]}
    shared["triu"] = triu
    shared["ones_row"] = np.ones((1, 128), dtype=np.float32)
    onesred = np.zeros((128, 2), dtype=np.float32); onesred[:, 0] = 1.0
    shared["ones_red"] = onesred
    sel_den = np.zeros((128, 4, 4), dtype=np.float32)
    for j in range(4):
        sel_den[:, j, j] = 1.0
    shared["sel_den"] = sel_den
    sel_bc2 = np.zeros((4, 2, 128), dtype=np.float32)
    for q in range(2):
        sel_bc2[2 * q, q, 0:64] = 1.0
        sel_bc2[2 * q + 1, q, 64:128] = 1.0
    shared["sel_bc2"] = sel_bc2
    shared["ones2d"] = np.ones((128, 128), dtype=np.float32)
    shared["ident"] = np.eye(128, dtype=np.float32)
    in_maps = []
    for b in range(N_CORES):
        x0 = we[tokens[b]] + pos                       # [S, D]
        m = dict(shared)
        m["x0T"] = np.ascontiguousarray(x0.T, dtype=np.float32)
        in_maps.append(m)
    return in_maps


def run(inputs, trace=False):
    nc = _get_program()
    in_maps = make_in_maps(inputs)
    res = bass_utils.run_bass_kernel_spmd(nc, in_maps,
                                          core_ids=list(range(N_CORES)),
                                          trace=trace)
    outs = np.stack([res.results[b]["out"].T for b in range(N_CORES)])
    return outs.astype(np.float32), res


def kernel(**inputs):
    out, _ = run(inputs, trace=False)
    return out


# revision 17
# speedup vs baseline: 1.2096x; 1.0112x over previous
"""Bass/Trainium2 kernel for a 12-layer GPT-style transformer (nn_BERT).

Strategy: data-parallel over batch (B=8 -> 1 sequence per NeuronCore).
Each core runs all 12 layers on x^T [D=768, S=512] in "transposed"
activation layout (feature dim on partitions), f32r matmul datapath.

kernel(**inputs) takes the FULL unsharded inputs (as produced by
reference.setup_inputs()) and returns the full [8, 512, 768] output.
"""
import contextlib
import os
import sys
import types

sys.path.insert(0, "/opt/trn_rl_repo")
os.environ.setdefault("JAX_PLATFORMS", "axon")

import numpy as np

import concourse.bass as bass
import concourse.mybir as mybir
import concourse.tile as tile
from concourse import bacc
from concourse import bass_utils

F32 = mybir.dt.float32
F32R = mybir.dt.float32r
AF = mybir.ActivationFunctionType
OP = mybir.AluOpType

B, S, D, H, L, V = 8, 512, 768, 12, 12, 40478
DH = D // H            # 64
DF = 4 * D             # 3072
KC = D // 128          # 6 chunks of the model dim
KF = DF // 128         # 24 chunks of the ffn dim
SC = S // 128          # 4 chunks of the sequence
EPS = 1e-5

N_CORES = 8


def _install_ntff_hook():
    """Register the axon NTFF profiling hook that this image's antenv lacks."""
    if "antenv.axon_hooks" in sys.modules:
        return
    try:
        mod = types.ModuleType("antenv.axon_hooks")
        _h = [None]
        mod.set_axon_ntff_profile_hook = lambda h: _h.__setitem__(0, h)
        mod.get_axon_ntff_profile_hook = lambda: _h[0]
        sys.modules["antenv.axon_hooks"] = mod
        import antenv

        antenv.axon_hooks = mod
        if "/root/.axon_site" not in sys.path:
            sys.path.insert(0, "/root/.axon_site")
        from trn_agent_boot.trn_boot import _ntff_profile_via_ctypes

        mod.set_axon_ntff_profile_hook(
            _ntff_profile_via_ctypes("/opt/axon/libaxon_pjrt.so")
        )
    except Exception:
        pass


def build_program(n_layers=L, phases="ABCLD"):
    nc = bacc.Bacc("TRN2", target_bir_lowering=False, debug=False,
                   num_devices=N_CORES)

    d = {}
    d["x0"] = nc.dram_tensor("x0T", (D, S), F32R, kind="ExternalInput").ap()
    d["wqkv"] = nc.dram_tensor("wqkv", (n_layers, D, 3 * D), F32R, kind="ExternalInput").ap()
    d["bqkv"] = nc.dram_tensor("bqkv", (n_layers, 3 * D), F32, kind="ExternalInput").ap()
    d["wproj"] = nc.dram_tensor("wproj", (n_layers, D, D), F32R, kind="ExternalInput").ap()
    d["bproj"] = nc.dram_tensor("bproj", (n_layers, D), F32, kind="ExternalInput").ap()
    d["g1"] = nc.dram_tensor("g1", (n_layers, D), F32, kind="ExternalInput").ap()
    d["b1"] = nc.dram_tensor("b1", (n_layers, D), F32, kind="ExternalInput").ap()
    d["wfc"] = nc.dram_tensor("wfc", (n_layers, D, DF), F32R, kind="ExternalInput").ap()
    d["bfc"] = nc.dram_tensor("bfc", (n_layers, DF), F32, kind="ExternalInput").ap()
    d["wpr"] = nc.dram_tensor("wpr", (n_layers, DF, D), F32R, kind="ExternalInput").ap()
    d["bpr"] = nc.dram_tensor("bpr", (n_layers, D), F32, kind="ExternalInput").ap()
    d["g2"] = nc.dram_tensor("g2", (n_layers, D), F32, kind="ExternalInput").ap()
    d["b2"] = nc.dram_tensor("b2", (n_layers, D), F32, kind="ExternalInput").ap()
    d["triu"] = nc.dram_tensor("triu", (128, 128), F32R, kind="ExternalInput").ap()
    d["ones_row"] = nc.dram_tensor("ones_row", (1, 128), F32R, kind="ExternalInput").ap()
    d["ones_red"] = nc.dram_tensor("ones_red", (128, 2), F32R, kind="ExternalInput").ap()
    d["sel_den"] = nc.dram_tensor("sel_den", (128, 4, 4), F32R, kind="ExternalInput").ap()
    d["sel_bc2"] = nc.dram_tensor("sel_bc2", (4, 2, 128), F32R, kind="ExternalInput").ap()
    d["ones2d"] = nc.dram_tensor("ones2d", (128, 128), F32R, kind="ExternalInput").ap()
    d["ident"] = nc.dram_tensor("ident", (128, 128), F32R, kind="ExternalInput").ap()
    d["out"] = nc.dram_tensor("out", (D, S), F32R, kind="ExternalOutput").ap()

    with tile.TileContext(nc) as tc, \
         nc.allow_low_precision(reason="f32r datapath; rel-err budget 2e-2"):
        _emit(tc, nc, n_layers, d, phases)
    nc.compile()
    return nc


def _emit(tc, nc, n_layers, d, phases="ABCLD"):
    ctx = contextlib.ExitStack()

    consts = ctx.enter_context(tc.tile_pool(name="consts", bufs=1))
    uni = ctx.enter_context(tc.tile_pool(name="uni", bufs=1))
    x_pool = ctx.enter_context(tc.tile_pool(name="xp", bufs=2))
    probs_pool = ctx.enter_context(tc.tile_pool(name="probs", bufs=2))
    stats_pool = ctx.enter_context(tc.tile_pool(name="stats", bufs=2))
    stats1_pool = ctx.enter_context(tc.tile_pool(name="stats1", bufs=1))
    wq_pool = ctx.enter_context(tc.tile_pool(name="wq", bufs=2))
    wp_pool = ctx.enter_context(tc.tile_pool(name="wp", bufs=1))
    wf_pool = ctx.enter_context(tc.tile_pool(name="wf", bufs=2))
    wr_pool = ctx.enter_context(tc.tile_pool(name="wr", bufs=3))
    gelu_pool = ctx.enter_context(tc.tile_pool(name="gelu", bufs=3))
    bias_pool = ctx.enter_context(tc.tile_pool(name="bias", bufs=1))

    cn = {}
    cn["triu"] = consts.tile([128, 128], F32R, tag="triu", name="triu")
    nc.sync.dma_start(out=cn["triu"], in_=d["triu"])
    cn["ones1"] = consts.tile([1, 128], F32R, tag="ones1", name="ones1")
    nc.sync.dma_start(out=cn["ones1"], in_=d["ones_row"])
    cn["ones_red"] = consts.tile([128, 2], F32R, tag="ones_red", name="ones_red")
    nc.sync.dma_start(out=cn["ones_red"], in_=d["ones_red"])
    cn["sel_den"] = consts.tile([128, 4, 4], F32R, tag="sel_den", name="sel_den")
    nc.sync.dma_start(out=cn["sel_den"], in_=d["sel_den"])
    cn["sel_bc2"] = consts.tile([4, 2, 128], F32R, tag="sel_bc2", name="sel_bc2")
    nc.sync.dma_start(out=cn["sel_bc2"], in_=d["sel_bc2"])
    cn["ones2d"] = consts.tile([128, 128], F32R, tag="ones2d", name="ones2d")
    nc.sync.dma_start(out=cn["ones2d"], in_=d["ones2d"])
    cn["ident"] = consts.tile([128, 128], F32R, tag="ident", name="ident")
    nc.sync.dma_start(out=cn["ident"], in_=d["ident"])
    cn["eps"] = consts.tile([1, 1], F32, tag="eps", name="eps")
    nc.vector.memset(cn["eps"], EPS)

    pools = dict(uni=uni, x=x_pool, probs=probs_pool, stats=stats_pool,
                 stats1=stats1_pool, wq=wq_pool, wp=wp_pool, wf=wf_pool,
                 wr=wr_pool, gelu=gelu_pool, bias=bias_pool)

    # residual stream x^T as per-chunk tiles [128, S]
    x_cur = []
    for k in range(KC):
        xk = x_pool.tile([128, S], F32R, tag=f"x{k}", name=f"x{k}")
        nc.sync.dma_start(out=xk, in_=d["x0"][128 * k:128 * (k + 1), :])
        x_cur.append(xk)

    for l in range(n_layers):
        with nc.named_scope(f"layer{l}"):
            x_cur = _layer(tc, nc, l, x_cur, d, cn, pools, phases)

    for k in range(KC):
        nc.sync.dma_start(out=d["out"][128 * k:128 * (k + 1), :], in_=x_cur[k])
    ctx.close()


def _ld_bias(nc, pool, dram_ap, tag, width):
    t = pool.tile([128, width], F32, tag=tag, name=tag)
    nc.sync.dma_start(out=t, in_=dram_ap.rearrange("(c p) -> p c", p=128))
    return t


def _layer(tc, nc, l, x_cur, d, cn, pools, phases="ABCLD"):
    uni = pools["uni"]; stats_pool = pools["stats"]; bias_pool = pools["bias"]
    stats1_pool = pools["stats1"]

    bqkv_t = _ld_bias(nc, bias_pool, d["bqkv"][l], "bqkv", 3 * D // 128)
    bv_b = bias_pool.tile([128, D], F32, tag="bvb", name="bvb")
    nc.sync.dma_start(out=bv_b, in_=d["bqkv"][l, 2 * D:3 * D].partition_broadcast(128))
    bproj_t = _ld_bias(nc, bias_pool, d["bproj"][l], "bproj", KC)
    g1_t = _ld_bias(nc, bias_pool, d["g1"][l], "g1", KC)
    b1_t = _ld_bias(nc, bias_pool, d["b1"][l], "b1", KC)
    bfc_t = _ld_bias(nc, bias_pool, d["bfc"][l], "bfc", KF)
    bpr_t = _ld_bias(nc, bias_pool, d["bpr"][l], "bpr", KC)
    g2_t = _ld_bias(nc, bias_pool, d["g2"][l], "g2", KC)
    b2_t = _ld_bias(nc, bias_pool, d["b2"][l], "b2", KC)

    # =====================================================================
    # Phase A: qkv.  Per-chunk tiles; wqkv streamed in 6 column-parts.
    # =====================================================================
    qT = [uni.tile([128, S], F32R, tag=f"u_q{k}", name=f"qT{k}") for k in range(KC)]
    kT = [uni.tile([128, S], F32R, tag=f"u_k{k}", name=f"kT{k}") for k in range(KC)]
    v_nat = [uni.tile([128, H, DH], F32R, tag=f"u_v{c}", name=f"vnat{c}")
             for c in range(SC)]

    with tc.tile_pool(name="ps_qk", bufs=3, space="PSUM") as ps_qk, \
         tc.tile_pool(name="ps_v", bufs=2, space="PSUM") as ps_v:
        for p in range(4):
            wpart = pools["wq"].tile([128, KC, 384], F32R, tag="wqkv")
            nc.sync.dma_start(
                out=wpart,
                in_=d["wqkv"][l, :, 384 * p:384 * (p + 1)].rearrange(
                    "(k q) n -> q k n", q=128))
            for j in range(3):
                oc = 3 * p + j
                pt = ps_qk.tile([128, S], F32)
                for k in range(KC):
                    nc.tensor.matmul(pt, wpart[:, k, 128 * j:128 * (j + 1)],
                                     x_cur[k], start=(k == 0),
                                     stop=(k == KC - 1))
                dst = qT[oc] if oc < KC else kT[oc - KC]
                nc.vector.tensor_scalar(out=dst, in0=pt,
                                        scalar1=bqkv_t[:, oc:oc + 1],
                                        scalar2=None, op0=OP.add)
        for p in range(4, 6):
            wpart = pools["wq"].tile([128, KC, 384], F32R, tag="wqkv")
            nc.sync.dma_start(
                out=wpart,
                in_=d["wqkv"][l, :, 384 * p:384 * (p + 1)].rearrange(
                    "(k q) n -> q k n", q=128))
            n0 = 384 * (p - 4)
            h0 = n0 // DH
            for sc in range(SC):
                pv = ps_v.tile([128, 384], F32, tag="pv")
                for k in range(KC):
                    nc.tensor.matmul(pv, x_cur[k][:, 128 * sc:128 * (sc + 1)],
                                     wpart[:, k, :], start=(k == 0),
                                     stop=(k == KC - 1))
                nc.vector.tensor_tensor(
                    out=v_nat[sc][:, h0:h0 + 6, :],
                    in0=pv.rearrange("q (h e) -> q h e", e=DH),
                    in1=bv_b[:, n0:n0 + 384].rearrange("q (h e) -> q h e", e=DH),
                    op=OP.add)

    if "B" not in phases:
        return qT
    # =====================================================================
    # Phase B: attention.  Per-chunk probs tiles; grouped reciprocals.
    # =====================================================================
    aT = [uni.tile([128, S], F32R, tag=f"u_a{j}", name=f"aT{j}") for j in range(KC)]
    G = 4
    with tc.tile_pool(name="ps_sc", bufs=2, space="PSUM") as ps_sc, \
         tc.tile_pool(name="ps_av", bufs=1, space="PSUM") as ps_av, \
         tc.tile_pool(name="ps_dn", bufs=1, space="PSUM") as ps_dn, \
         tc.tile_pool(name="ps_bc", bufs=1, space="PSUM") as ps_bc:
        for g in range(H // G):
            pden = ps_dn.tile([G, S], F32, tag="den")
            pavs = []
            for j in range(G):
                h = G * g + j
                hc, hh = h // 2, (h % 2) * 64
                probs = [pools["probs"].tile([128, S], F32R, tag=f"pb{c}",
                                             name=f"pb{c}") for c in range(SC)]
                for c in range(SC):
                    n0 = 128 * c if c < SC - 1 else 256
                    pt = ps_sc.tile([128, S], F32, tag="score")
                    nc.tensor.matmul(pt[:, 0:S - n0],
                                     kT[hc][hh:hh + 64, 128 * c:128 * c + 128],
                                     qT[hc][hh:hh + 64, n0:S],
                                     start=True, stop=True)
                    nc.scalar.activation(out=probs[c][:, n0:S],
                                         in_=pt[:, 0:S - n0],
                                         func=AF.Exp, scale=0.125)
                    if c == SC - 1:
                        nc.vector.tensor_scalar(out=probs[c][:, 256:384],
                                                in0=probs[c][:, 256:384],
                                                scalar1=0.0, scalar2=None,
                                                op0=OP.mult)
                    nc.vector.tensor_tensor(
                        out=probs[c][:, 128 * c:128 * c + 128],
                        in0=probs[c][:, 128 * c:128 * c + 128],
                        in1=cn["triu"], op=OP.mult)
                for c in range(SC):
                    n0 = 128 * c if c < SC - 1 else 256
                    nc.tensor.matmul(pden[:, n0:S], cn["sel_den"][:, j, :],
                                     probs[c][:, n0:S],
                                     start=(j == 0 and c == 0),
                                     stop=(j == G - 1 and c == SC - 1),
                                     skip_group_check=True)
                if hh == 0:
                    pav = ps_av.tile([64, S], F32, tag=f"av_e{j // 2}",
                                     name=f"pav_e{j // 2}")
                else:
                    pav = ps_av.tile([128, S], F32, tag=f"av_o{j // 2}",
                                     name=f"pav_o{j // 2}")
                pavs.append(pav)
                for c in range(SC):
                    n0 = 128 * c if c < SC - 1 else 256
                    if hh == 0:
                        lt = v_nat[c][:, h, :]
                        dst = pav[0:64, n0:S]
                    else:
                        lt = v_nat[c][:, h - 1:h + 1, :].rearrange(
                            "p h e -> p (h e)")
                        dst = pav[0:128, n0:S]
                    nc.tensor.matmul(dst, lt, probs[c][:, n0:S],
                                     start=(c == 0), stop=(c == SC - 1),
                                     skip_group_check=True)
            recip32 = stats1_pool.tile([G, S], F32, tag="recip32")
            nc.vector.reciprocal_approx_fast(out=recip32, in_=pden[0:G, :])
            recip_r = stats1_pool.tile([G, S], F32R, tag="recipr")
            nc.vector.tensor_copy(out=recip_r, in_=recip32)
            for q in range(2):
                hc = 2 * g + q
                pbc = ps_bc.tile([128, S], F32, tag="bc")
                nc.tensor.matmul(pbc, cn["sel_bc2"][:, q, :], recip_r,
                                 start=True, stop=True)
                bc_s = stats_pool.tile([128, S], F32, tag="bc_s")
                nc.vector.tensor_copy(out=bc_s, in_=pbc)
                nc.vector.tensor_tensor(out=aT[hc][0:64, :],
                                        in0=pavs[2 * q][0:64, :],
                                        in1=bc_s[0:64, :], op=OP.mult)
                nc.vector.tensor_tensor(out=aT[hc][64:128, :],
                                        in0=pavs[2 * q + 1][64:128, :],
                                        in1=bc_s[64:128, :], op=OP.mult)

    if "C" not in phases:
        return aT
    # =====================================================================
    # Phase C: attn out proj + residual (on PE) + bias (ScalarE)
    # =====================================================================
    wproj_t = pools["wp"].tile([128, KC, D], F32R, tag="wproj")
    nc.sync.dma_start(out=wproj_t,
                      in_=d["wproj"][l].rearrange("(k p) n -> p k n", p=128))
    res1 = [uni.tile([128, S], F32R, tag=f"u_k{k}", name=f"res1_{k}")
            for k in range(KC)]
    with tc.tile_pool(name="ps_pj", bufs=3, space="PSUM") as ps_pj:
        for oc in range(KC):
            pt = ps_pj.tile([128, S], F32)
            for k in range(KC):
                nc.tensor.matmul(pt, wproj_t[:, k, 128 * oc:128 * (oc + 1)],
                                 aT[k], start=(k == 0), stop=False)
            nc.tensor.matmul(pt, cn["ident"], x_cur[oc],
                             start=False, stop=True)
            nc.scalar.activation(out=res1[oc], in_=pt, func=AF.Identity,
                                 bias=bproj_t[:, oc:oc + 1], scale=1.0)

    if "L" not in phases:
        return res1
    nT = [uni.tile([128, S], F32R, tag=f"u_q{k}", name=f"nT{k}")
          for k in range(KC)]
    _layernorm(tc, nc, res1, nT, g1_t, b1_t, cn, stats_pool, stats1_pool, "ln1")

    if "D" not in phases:
        return nT
    # =====================================================================
    # Phase D: fused fc -> gelu -> pr; residual via identity matmul.
    # =====================================================================
    res2 = [uni.tile([128, S], F32R, tag=f"u_a{k}", name=f"res2_{k}")
            for k in range(KC)]
    with tc.tile_pool(name="ps_pr", bufs=1, space="PSUM") as ps_pr, \
         tc.tile_pool(name="ps_fc", bufs=2, space="PSUM") as ps_fc:
        pr_acc = [ps_pr.tile([128, S], F32, tag=f"pr{oc}", name=f"pr{oc}")
                  for oc in range(KC)]
        for part in range(6):
            wfc_p = pools["wf"].tile([128, KC, 512], F32R, tag="wfc")
            nc.sync.dma_start(
                out=wfc_p,
                in_=d["wfc"][l, :, 512 * part:512 * (part + 1)].rearrange(
                    "(k q) n -> q k n", q=128))
            for j in range(4):
                kf = 4 * part + j
                wpr_k = pools["wr"].tile([128, D], F32R, tag="wprk")
                nc.sync.dma_start(out=wpr_k,
                                  in_=d["wpr"][l, 128 * kf:128 * (kf + 1), :])
                pf = ps_fc.tile([128, S], F32)
                for k in range(KC):
                    nc.tensor.matmul(pf, wfc_p[:, k, 128 * j:128 * (j + 1)],
                                     nT[k], start=(k == 0),
                                     stop=(k == KC - 1))
                gk = pools["gelu"].tile([128, S], F32R, tag="gk")
                nc.scalar.activation(out=gk, in_=pf, func=AF.Gelu_apprx_tanh,
                                     bias=bfc_t[:, kf:kf + 1], scale=1.0)
                for oc in range(KC):
                    nc.tensor.matmul(pr_acc[oc],
                                     wpr_k[:, 128 * oc:128 * (oc + 1)],
                                     gk, start=(kf == 0), stop=False)
        for oc in range(KC):
            nc.tensor.matmul(pr_acc[oc], cn["ident"], nT[oc],
                             start=False, stop=True)
            nc.scalar.activation(out=res2[oc], in_=pr_acc[oc],
                                 func=AF.Identity,
                                 bias=bpr_t[:, oc:oc + 1], scale=1.0)

    x_next = [pools["x"].tile([128, S], F32R, tag=f"x{k}", name=f"xn{k}")
              for k in range(KC)]
    _layernorm(tc, nc, res2, x_next, g2_t, b2_t, cn, stats_pool, stats1_pool,
               "ln2")
    return x_next


def _layernorm(tc, nc, src, dst, g_t, b_t, cn, stats_pool, stats1_pool, tag):
    """LN over the partition (feature) axis; src/dst are per-chunk tiles."""
    with tc.tile_pool(name=f"ps_{tag}", bufs=1, space="PSUM") as ps:
        psums = ps.tile([128, S], F32, tag="bsum")   # every row = sum(x)
        psq = ps.tile([2, S], F32, tag="s1")         # row 0: sum(x^2)
        for k in range(KC):
            sq = stats_pool.tile([128, S], F32R, tag="lnsq")
            nc.scalar.activation(out=sq, in_=src[k], func=AF.Square)
            nc.tensor.matmul(psums, cn["ones2d"], src[k],
                             start=(k == 0), stop=(k == KC - 1))
            nc.tensor.matmul(psq, cn["ones_red"], sq,
                             start=(k == 0), stop=(k == KC - 1))
        bsum_s = stats1_pool.tile([128, S], F32, tag="bsum_s")
        nc.vector.tensor_copy(out=bsum_s, in_=psums)
        mu1 = stats1_pool.tile([1, S], F32, tag="mu1")
        var = stats1_pool.tile([1, S], F32, tag="var")
        rsd = stats1_pool.tile([1, S], F32R, tag="rsd")
        nc.vector.tensor_scalar(out=mu1, in0=bsum_s[0:1, :], scalar1=1.0 / D,
                                scalar2=None, op0=OP.mult)
        nc.vector.tensor_tensor(out=var, in0=mu1, in1=mu1, op=OP.mult)
        nc.vector.scalar_tensor_tensor(out=var, in0=psq[0:1, :], scalar=1.0 / D,
                                       in1=var, op0=OP.mult, op1=OP.subtract)
        nc.scalar.activation(out=var, in_=var, func=AF.Sqrt, bias=cn["eps"])
        rsd32 = stats1_pool.tile([1, S], F32, tag="rsd32")
        nc.vector.reciprocal_approx_fast(out=rsd32, in_=var)
        nc.vector.tensor_copy(out=rsd, in_=rsd32)
        prs = ps.tile([128, S], F32, tag="s1")       # reuse the psq bank
        nc.tensor.matmul(prs, cn["ones1"], rsd, start=True, stop=True)
        brs_s = stats1_pool.tile([128, S], F32, tag="brs_s")
        nc.vector.tensor_copy(out=brs_s, in_=prs)
        for k in range(KC):
            t = stats_pool.tile([128, S], F32, tag="lnt")
            nc.vector.scalar_tensor_tensor(out=t, in0=bsum_s, scalar=-1.0 / D,
                                           in1=src[k], op0=OP.mult,
                                           op1=OP.add)
            nc.vector.tensor_tensor(out=t, in0=t, in1=brs_s, op=OP.mult)
            nc.scalar.activation(out=dst[k], in_=t, func=AF.Identity,
                                 bias=b_t[:, k:k + 1],
                                 scale=g_t[:, k:k + 1])


# =========================================================================
# Host side
# =========================================================================
_CACHE = {}


def _get_program():
    if "nc" not in _CACHE:
        _install_ntff_hook()
        _CACHE["nc"] = build_program(L)
    return _CACHE["nc"]


def make_in_maps(inputs, n_layers=L):
    tokens = np.asarray(inputs["tokens"])
    we = np.asarray(inputs["we"], dtype=np.float32)
    pos = we[V:V + S]                                  # [S, D]
    triu = np.triu(np.ones((128, 128), dtype=np.float32))

    def f32(name):
        return np.ascontiguousarray(np.asarray(inputs[name])[:n_layers],
                                    dtype=np.float32)

    shared = {k: f32(k) for k in ["wqkv", "bqkv", "wproj", "bproj", "g1", "b1",
                                  "wfc", "bfc", "wpr", "bpr", "g2", "b2"]}
    shared["triu"] = triu
    shared["ones_row"] = np.ones((1, 128), dtype=np.float32)
    onesred = np.zeros((128, 2), dtype=np.float32); onesred[:, 0] = 1.0
    shared["ones_red"] = onesred
    sel_den = np.zeros((128, 4, 4), dtype=np.float32)
    for j in range(4):
        sel_den[:, j, j] = 1.0
    shared["sel_den"] = sel_den
    sel_bc2 = np.zeros((4, 2, 128), dtype=np.float32)
    for q in range(2):
        sel_bc2[2 * q, q, 0:64] = 1.0
        sel_bc2[2 * q + 1, q, 64:128] = 1.0
    shared["sel_bc2"] = sel_bc2
    shared["ones2d"] = np.ones((128, 128), dtype=np.float32)
    shared["ident"] = np.eye(128, dtype=np.float32)
    in_maps = []
    for b in range(N_CORES):
        x0 = we[tokens[b]] + pos                       # [S, D]
        m = dict(shared)
        m["x0T"] = np.ascontiguousarray(x0.T, dtype=np.float32)
        in_maps.append(m)
    return in_maps


def run(inputs, trace=False):
    nc = _get_program()
    in_maps = make_in_maps(inputs)
    res = bass_utils.run_bass_kernel_spmd(nc, in_maps,
                                          core_ids=list(range(N_CORES)),
                                          trace=trace)
    outs = np.stack([res.results[b]["out"].T for b in range(N_CORES)])
    return outs.astype(np.float32), res


def kernel(**inputs):
    out, _ = run(inputs, trace=False)
    return out


# revision 19
# speedup vs baseline: 1.2753x; 1.0543x over previous
"""Bass/Trainium2 kernel for a 12-layer GPT-style transformer (nn_BERT).

Strategy: data-parallel over batch (B=8 -> 1 sequence per NeuronCore).
Each core runs all 12 layers on x^T [D=768, S=512] in "transposed"
activation layout (feature dim on partitions), f32r matmul datapath.

kernel(**inputs) takes the FULL unsharded inputs (as produced by
reference.setup_inputs()) and returns the full [8, 512, 768] output.
"""
import contextlib
import os
import sys
import types

sys.path.insert(0, "/opt/trn_rl_repo")
os.environ.setdefault("JAX_PLATFORMS", "axon")

import numpy as np

import concourse.bass as bass
import concourse.mybir as mybir
import concourse.tile as tile
from concourse import bacc
from concourse import bass_utils

F32 = mybir.dt.float32
F32R = mybir.dt.float32r
AF = mybir.ActivationFunctionType
OP = mybir.AluOpType

B, S, D, H, L, V = 8, 512, 768, 12, 12, 40478
DH = D // H            # 64
DF = 4 * D             # 3072
KC = D // 128          # 6 chunks of the model dim
KF = DF // 128         # 24 chunks of the ffn dim
SC = S // 128          # 4 chunks of the sequence
EPS = 1e-5

N_CORES = 8


def _install_ntff_hook():
    """Register the axon NTFF profiling hook that this image's antenv lacks."""
    if "antenv.axon_hooks" in sys.modules:
        return
    try:
        mod = types.ModuleType("antenv.axon_hooks")
        _h = [None]
        mod.set_axon_ntff_profile_hook = lambda h: _h.__setitem__(0, h)
        mod.get_axon_ntff_profile_hook = lambda: _h[0]
        sys.modules["antenv.axon_hooks"] = mod
        import antenv

        antenv.axon_hooks = mod
        if "/root/.axon_site" not in sys.path:
            sys.path.insert(0, "/root/.axon_site")
        from trn_agent_boot.trn_boot import _ntff_profile_via_ctypes

        mod.set_axon_ntff_profile_hook(
            _ntff_profile_via_ctypes("/opt/axon/libaxon_pjrt.so")
        )
    except Exception:
        pass


def build_program(n_layers=L, phases="ABCLD"):
    nc = bacc.Bacc("TRN2", target_bir_lowering=False, debug=False,
                   num_devices=N_CORES)

    d = {}
    d["x0"] = nc.dram_tensor("x0T", (D, S), F32R, kind="ExternalInput").ap()
    d["wqkv"] = nc.dram_tensor("wqkv", (n_layers, D, 3 * D), F32R, kind="ExternalInput").ap()
    d["bqkv"] = nc.dram_tensor("bqkv", (n_layers, 3 * D), F32, kind="ExternalInput").ap()
    d["wproj"] = nc.dram_tensor("wproj", (n_layers, D, D), F32R, kind="ExternalInput").ap()
    d["bproj"] = nc.dram_tensor("bproj", (n_layers, D), F32, kind="ExternalInput").ap()
    d["g1"] = nc.dram_tensor("g1", (n_layers, D), F32, kind="ExternalInput").ap()
    d["b1"] = nc.dram_tensor("b1", (n_layers, D), F32, kind="ExternalInput").ap()
    d["wfc"] = nc.dram_tensor("wfc", (n_layers, D, DF), F32R, kind="ExternalInput").ap()
    d["bfc"] = nc.dram_tensor("bfc", (n_layers, DF), F32, kind="ExternalInput").ap()
    d["wpr"] = nc.dram_tensor("wpr", (n_layers, DF, D), F32R, kind="ExternalInput").ap()
    d["bpr"] = nc.dram_tensor("bpr", (n_layers, D), F32, kind="ExternalInput").ap()
    d["g2"] = nc.dram_tensor("g2", (n_layers, D), F32, kind="ExternalInput").ap()
    d["b2"] = nc.dram_tensor("b2", (n_layers, D), F32, kind="ExternalInput").ap()
    d["triu"] = nc.dram_tensor("triu", (128, 128), F32R, kind="ExternalInput").ap()
    d["ones_row"] = nc.dram_tensor("ones_row", (1, 128), F32R, kind="ExternalInput").ap()
    d["ones_red"] = nc.dram_tensor("ones_red", (128, 2), F32R, kind="ExternalInput").ap()
    d["sel_den"] = nc.dram_tensor("sel_den", (128, 4, 4), F32R, kind="ExternalInput").ap()
    d["sel_bc2"] = nc.dram_tensor("sel_bc2", (4, 2, 128), F32R, kind="ExternalInput").ap()
    d["ones2d"] = nc.dram_tensor("ones2d", (128, 128), F32R, kind="ExternalInput").ap()
    d["ident"] = nc.dram_tensor("ident", (128, 128), F32R, kind="ExternalInput").ap()
    d["out"] = nc.dram_tensor("out", (D, S), F32R, kind="ExternalOutput").ap()

    with tile.TileContext(nc) as tc, \
         nc.allow_low_precision(reason="f32r datapath; rel-err budget 2e-2"):
        _emit(tc, nc, n_layers, d, phases)
    nc.compile()
    return nc


def _emit(tc, nc, n_layers, d, phases="ABCLD"):
    ctx = contextlib.ExitStack()

    consts = ctx.enter_context(tc.tile_pool(name="consts", bufs=1))
    uni = ctx.enter_context(tc.tile_pool(name="uni", bufs=1))
    x_pool = ctx.enter_context(tc.tile_pool(name="xp", bufs=2))
    probs_pool = ctx.enter_context(tc.tile_pool(name="probs", bufs=3))
    stats_pool = ctx.enter_context(tc.tile_pool(name="stats", bufs=2))
    stats1_pool = ctx.enter_context(tc.tile_pool(name="stats1", bufs=1))
    wq_pool = ctx.enter_context(tc.tile_pool(name="wq", bufs=2))
    wp_pool = ctx.enter_context(tc.tile_pool(name="wp", bufs=1))
    wf_pool = ctx.enter_context(tc.tile_pool(name="wf", bufs=2))
    wr_pool = ctx.enter_context(tc.tile_pool(name="wr", bufs=3))
    gelu_pool = ctx.enter_context(tc.tile_pool(name="gelu", bufs=3))
    bias_pool = ctx.enter_context(tc.tile_pool(name="bias", bufs=1))

    cn = {}
    cn["triu"] = consts.tile([128, 128], F32R, tag="triu", name="triu")
    nc.sync.dma_start(out=cn["triu"], in_=d["triu"])
    cn["ones1"] = consts.tile([1, 128], F32R, tag="ones1", name="ones1")
    nc.sync.dma_start(out=cn["ones1"], in_=d["ones_row"])
    cn["ones_red"] = consts.tile([128, 2], F32R, tag="ones_red", name="ones_red")
    nc.sync.dma_start(out=cn["ones_red"], in_=d["ones_red"])
    cn["sel_den"] = consts.tile([128, 4, 4], F32R, tag="sel_den", name="sel_den")
    nc.sync.dma_start(out=cn["sel_den"], in_=d["sel_den"])
    cn["sel_bc2"] = consts.tile([4, 2, 128], F32R, tag="sel_bc2", name="sel_bc2")
    nc.sync.dma_start(out=cn["sel_bc2"], in_=d["sel_bc2"])
    cn["ones2d"] = consts.tile([128, 128], F32R, tag="ones2d", name="ones2d")
    nc.sync.dma_start(out=cn["ones2d"], in_=d["ones2d"])
    cn["ident"] = consts.tile([128, 128], F32R, tag="ident", name="ident")
    nc.sync.dma_start(out=cn["ident"], in_=d["ident"])
    cn["eps"] = consts.tile([1, 1], F32, tag="eps", name="eps")
    nc.vector.memset(cn["eps"], EPS)

    pools = dict(uni=uni, x=x_pool, probs=probs_pool, stats=stats_pool,
                 stats1=stats1_pool, wq=wq_pool, wp=wp_pool, wf=wf_pool,
                 wr=wr_pool, gelu=gelu_pool, bias=bias_pool)

    # residual stream x^T as per-chunk tiles [128, S]
    x_cur = []
    for k in range(KC):
        xk = x_pool.tile([128, S], F32R, tag=f"x{k}", name=f"x{k}")
        nc.sync.dma_start(out=xk, in_=d["x0"][128 * k:128 * (k + 1), :])
        x_cur.append(xk)

    for l in range(n_layers):
        with nc.named_scope(f"layer{l}"):
            x_cur = _layer(tc, nc, l, x_cur, d, cn, pools, phases)

    for k in range(KC):
        nc.sync.dma_start(out=d["out"][128 * k:128 * (k + 1), :], in_=x_cur[k])
    ctx.close()


def _ld_bias(nc, pool, dram_ap, tag, width):
    t = pool.tile([128, width], F32, tag=tag, name=tag)
    nc.sync.dma_start(out=t, in_=dram_ap.rearrange("(c p) -> p c", p=128))
    return t


def _layer(tc, nc, l, x_cur, d, cn, pools, phases="ABCLD"):
    uni = pools["uni"]; stats_pool = pools["stats"]; bias_pool = pools["bias"]
    stats1_pool = pools["stats1"]

    bqkv_t = _ld_bias(nc, bias_pool, d["bqkv"][l], "bqkv", 3 * D // 128)
    bv_b = bias_pool.tile([128, D], F32, tag="bvb", name="bvb")
    nc.sync.dma_start(out=bv_b, in_=d["bqkv"][l, 2 * D:3 * D].partition_broadcast(128))
    bproj_t = _ld_bias(nc, bias_pool, d["bproj"][l], "bproj", KC)
    g1_t = _ld_bias(nc, bias_pool, d["g1"][l], "g1", KC)
    b1_t = _ld_bias(nc, bias_pool, d["b1"][l], "b1", KC)
    bfc_t = _ld_bias(nc, bias_pool, d["bfc"][l], "bfc", KF)
    bpr_t = _ld_bias(nc, bias_pool, d["bpr"][l], "bpr", KC)
    g2_t = _ld_bias(nc, bias_pool, d["g2"][l], "g2", KC)
    b2_t = _ld_bias(nc, bias_pool, d["b2"][l], "b2", KC)

    # =====================================================================
    # Phase A: qkv.  Per-chunk tiles; wqkv streamed in 6 column-parts.
    # =====================================================================
    qT = [uni.tile([128, S], F32R, tag=f"u_q{k}", name=f"qT{k}") for k in range(KC)]
    kT = [uni.tile([128, S], F32R, tag=f"u_k{k}", name=f"kT{k}") for k in range(KC)]
    v_nat = [uni.tile([128, H, DH], F32R, tag=f"u_v{c}", name=f"vnat{c}")
             for c in range(SC)]

    with tc.tile_pool(name="ps_qk", bufs=3, space="PSUM") as ps_qk, \
         tc.tile_pool(name="ps_v", bufs=2, space="PSUM") as ps_v:
        for p in range(4):
            wpart = pools["wq"].tile([128, KC, 384], F32R, tag="wqkv")
            nc.sync.dma_start(
                out=wpart,
                in_=d["wqkv"][l, :, 384 * p:384 * (p + 1)].rearrange(
                    "(k q) n -> q k n", q=128))
            for j in range(3):
                oc = 3 * p + j
                pt = ps_qk.tile([128, S], F32)
                for k in range(KC):
                    nc.tensor.matmul(pt, wpart[:, k, 128 * j:128 * (j + 1)],
                                     x_cur[k], start=(k == 0),
                                     stop=(k == KC - 1))
                dst = qT[oc] if oc < KC else kT[oc - KC]
                nc.vector.tensor_scalar(out=dst, in0=pt,
                                        scalar1=bqkv_t[:, oc:oc + 1],
                                        scalar2=None, op0=OP.add)
        for p in range(4, 6):
            wpart = pools["wq"].tile([128, KC, 384], F32R, tag="wqkv")
            nc.sync.dma_start(
                out=wpart,
                in_=d["wqkv"][l, :, 384 * p:384 * (p + 1)].rearrange(
                    "(k q) n -> q k n", q=128))
            n0 = 384 * (p - 4)
            h0 = n0 // DH
            for sc in range(SC):
                pv = ps_v.tile([128, 384], F32, tag="pv")
                for k in range(KC):
                    nc.tensor.matmul(pv, x_cur[k][:, 128 * sc:128 * (sc + 1)],
                                     wpart[:, k, :], start=(k == 0),
                                     stop=(k == KC - 1))
                nc.vector.tensor_tensor(
                    out=v_nat[sc][:, h0:h0 + 6, :],
                    in0=pv.rearrange("q (h e) -> q h e", e=DH),
                    in1=bv_b[:, n0:n0 + 384].rearrange("q (h e) -> q h e", e=DH),
                    op=OP.add)

    if "B" not in phases:
        return qT
    # =====================================================================
    # Phase B: attention.  Per-chunk probs tiles; grouped reciprocals.
    # =====================================================================
    aT = [uni.tile([128, S], F32R, tag=f"u_a{j}", name=f"aT{j}") for j in range(KC)]
    G = 4
    with tc.tile_pool(name="ps_sc", bufs=3, space="PSUM") as ps_sc, \
         tc.tile_pool(name="ps_av", bufs=1, space="PSUM") as ps_av, \
         tc.tile_pool(name="ps_dn", bufs=1, space="PSUM") as ps_dn:
        for g in range(H // G):
            pden = ps_dn.tile([G, S], F32, tag="den")
            pavs = []
            for j in range(G):
                h = G * g + j
                hc, hh = h // 2, (h % 2) * 64
                probs = [pools["probs"].tile([128, S], F32R, tag=f"pb{c}",
                                             name=f"pb{c}") for c in range(SC)]
                for c in range(SC):
                    n0 = 128 * c if c < SC - 1 else 256
                    pt = ps_sc.tile([128, S], F32, tag="score")
                    nc.tensor.matmul(pt[:, 0:S - n0],
                                     kT[hc][hh:hh + 64, 128 * c:128 * c + 128],
                                     qT[hc][hh:hh + 64, n0:S],
                                     start=True, stop=True)
                    nc.scalar.activation(out=probs[c][:, n0:S],
                                         in_=pt[:, 0:S - n0],
                                         func=AF.Exp, scale=0.125)
                    if c == SC - 1:
                        nc.vector.tensor_scalar(out=probs[c][:, 256:384],
                                                in0=probs[c][:, 256:384],
                                                scalar1=0.0, scalar2=None,
                                                op0=OP.mult)
                    nc.vector.tensor_tensor(
                        out=probs[c][:, 128 * c:128 * c + 128],
                        in0=probs[c][:, 128 * c:128 * c + 128],
                        in1=cn["triu"], op=OP.mult)
                for c in range(SC):
                    n0 = 128 * c if c < SC - 1 else 256
                    nc.tensor.matmul(pden[:, n0:S], cn["sel_den"][:, j, :],
                                     probs[c][:, n0:S],
                                     start=(j == 0 and c == 0),
                                     stop=(j == G - 1 and c == SC - 1),
                                     skip_group_check=True)
                if hh == 0:
                    pav = ps_av.tile([64, S], F32, tag=f"av_e{j // 2}",
                                     name=f"pav_e{j // 2}")
                else:
                    pav = ps_av.tile([128, S], F32, tag=f"av_o{j // 2}",
                                     name=f"pav_o{j // 2}")
                pavs.append(pav)
                for c in range(SC):
                    n0 = 128 * c if c < SC - 1 else 256
                    if hh == 0:
                        lt = v_nat[c][:, h, :]
                        dst = pav[0:64, n0:S]
                    else:
                        lt = v_nat[c][:, h - 1:h + 1, :].rearrange(
                            "p h e -> p (h e)")
                        dst = pav[0:128, n0:S]
                    nc.tensor.matmul(dst, lt, probs[c][:, n0:S],
                                     start=(c == 0), stop=(c == SC - 1),
                                     skip_group_check=True)
            recip32 = stats1_pool.tile([G, S], F32, tag="recip32")
            nc.vector.reciprocal_approx_fast(out=recip32, in_=pden[0:G, :])
            recip_r = stats1_pool.tile([G, S], F32R, tag="recipr")
            nc.vector.tensor_copy(out=recip_r, in_=recip32)
            for q in range(2):
                hc = 2 * g + q
                pbc = ps_dn.tile([128, S], F32, tag="den", name="pbc")
                nc.tensor.matmul(pbc, cn["sel_bc2"][:, q, :], recip_r,
                                 start=True, stop=True)
                bc_s = stats_pool.tile([128, S], F32, tag="bc_s")
                nc.vector.tensor_copy(out=bc_s, in_=pbc)
                nc.vector.tensor_tensor(out=aT[hc][0:64, :],
                                        in0=pavs[2 * q][0:64, :],
                                        in1=bc_s[0:64, :], op=OP.mult)
                nc.vector.tensor_tensor(out=aT[hc][64:128, :],
                                        in0=pavs[2 * q + 1][64:128, :],
                                        in1=bc_s[64:128, :], op=OP.mult)

    if "C" not in phases:
        return aT
    # =====================================================================
    # Phase C: attn out proj + residual (on PE) + bias (ScalarE)
    # =====================================================================
    wproj_t = pools["wp"].tile([128, KC, D], F32R, tag="wproj")
    nc.sync.dma_start(out=wproj_t,
                      in_=d["wproj"][l].rearrange("(k p) n -> p k n", p=128))
    res1 = [uni.tile([128, S], F32R, tag=f"u_k{k}", name=f"res1_{k}")
            for k in range(KC)]
    with tc.tile_pool(name="ps_pj", bufs=3, space="PSUM") as ps_pj:
        for oc in range(KC):
            pt = ps_pj.tile([128, S], F32)
            for k in range(KC):
                nc.tensor.matmul(pt, wproj_t[:, k, 128 * oc:128 * (oc + 1)],
                                 aT[k], start=(k == 0), stop=False)
            nc.tensor.matmul(pt, cn["ident"], x_cur[oc],
                             start=False, stop=True)
            nc.scalar.activation(out=res1[oc], in_=pt, func=AF.Identity,
                                 bias=bproj_t[:, oc:oc + 1], scale=1.0)

    if "L" not in phases:
        return res1
    nT = [uni.tile([128, S], F32R, tag=f"u_q{k}", name=f"nT{k}")
          for k in range(KC)]
    _layernorm(tc, nc, res1, nT, g1_t, b1_t, cn, stats_pool, stats1_pool, "ln1")

    if "D" not in phases:
        return nT
    # =====================================================================
    # Phase D: fused fc -> gelu -> pr; residual via identity matmul.
    # =====================================================================
    res2 = [uni.tile([128, S], F32R, tag=f"u_a{k}", name=f"res2_{k}")
            for k in range(KC)]
    with tc.tile_pool(name="ps_pr", bufs=1, space="PSUM") as ps_pr, \
         tc.tile_pool(name="ps_fc", bufs=2, space="PSUM") as ps_fc:
        pr_acc = [ps_pr.tile([128, S], F32, tag=f"pr{oc}", name=f"pr{oc}")
                  for oc in range(KC)]
        for part in range(6):
            wfc_p = pools["wf"].tile([128, KC, 512], F32R, tag="wfc")
            nc.sync.dma_start(
                out=wfc_p,
                in_=d["wfc"][l, :, 512 * part:512 * (part + 1)].rearrange(
                    "(k q) n -> q k n", q=128))
            for j in range(4):
                kf = 4 * part + j
                wpr_k = pools["wr"].tile([128, D], F32R, tag="wprk")
                nc.sync.dma_start(out=wpr_k,
                                  in_=d["wpr"][l, 128 * kf:128 * (kf + 1), :])
                pf = ps_fc.tile([128, S], F32)
                for k in range(KC):
                    nc.tensor.matmul(pf, wfc_p[:, k, 128 * j:128 * (j + 1)],
                                     nT[k], start=(k == 0),
                                     stop=(k == KC - 1))
                gk = pools["gelu"].tile([128, S], F32R, tag="gk")
                nc.scalar.activation(out=gk, in_=pf, func=AF.Gelu_apprx_tanh,
                                     bias=bfc_t[:, kf:kf + 1], scale=1.0)
                for oc in range(KC):
                    nc.tensor.matmul(pr_acc[oc],
                                     wpr_k[:, 128 * oc:128 * (oc + 1)],
                                     gk, start=(kf == 0), stop=False)
        for oc in range(KC):
            nc.tensor.matmul(pr_acc[oc], cn["ident"], nT[oc],
                             start=False, stop=True)
            nc.scalar.activation(out=res2[oc], in_=pr_acc[oc],
                                 func=AF.Identity,
                                 bias=bpr_t[:, oc:oc + 1], scale=1.0)

    x_next = [pools["x"].tile([128, S], F32R, tag=f"x{k}", name=f"xn{k}")
              for k in range(KC)]
    _layernorm(tc, nc, res2, x_next, g2_t, b2_t, cn, stats_pool, stats1_pool,
               "ln2")
    return x_next


def _layernorm(tc, nc, src, dst, g_t, b_t, cn, stats_pool, stats1_pool, tag):
    """LN over the partition (feature) axis; src/dst are per-chunk tiles."""
    with tc.tile_pool(name=f"ps_{tag}", bufs=1, space="PSUM") as ps:
        psums = ps.tile([128, S], F32, tag="bsum")   # every row = sum(x)
        psq = ps.tile([2, S], F32, tag="s1")         # row 0: sum(x^2)
        for k in range(KC):
            sq = stats_pool.tile([128, S], F32R, tag="lnsq")
            nc.scalar.activation(out=sq, in_=src[k], func=AF.Square)
            nc.tensor.matmul(psums, cn["ones2d"], src[k],
                             start=(k == 0), stop=(k == KC - 1))
            nc.tensor.matmul(psq, cn["ones_red"], sq,
                             start=(k == 0), stop=(k == KC - 1))
        bsum_s = stats1_pool.tile([128, S], F32, tag="bsum_s")
        nc.vector.tensor_copy(out=bsum_s, in_=psums)
        mu1 = stats1_pool.tile([1, S], F32, tag="mu1")
        var = stats1_pool.tile([1, S], F32, tag="var")
        rsd = stats1_pool.tile([1, S], F32R, tag="rsd")
        nc.vector.tensor_scalar(out=mu1, in0=bsum_s[0:1, :], scalar1=1.0 / D,
                                scalar2=None, op0=OP.mult)
        nc.vector.tensor_tensor(out=var, in0=mu1, in1=mu1, op=OP.mult)
        nc.vector.scalar_tensor_tensor(out=var, in0=psq[0:1, :], scalar=1.0 / D,
                                       in1=var, op0=OP.mult, op1=OP.subtract)
        nc.scalar.activation(out=var, in_=var, func=AF.Sqrt, bias=cn["eps"])
        rsd32 = stats1_pool.tile([1, S], F32, tag="rsd32")
        nc.vector.reciprocal_approx_fast(out=rsd32, in_=var)
        nc.vector.tensor_copy(out=rsd, in_=rsd32)
        prs = ps.tile([128, S], F32, tag="s1")       # reuse the psq bank
        nc.tensor.matmul(prs, cn["ones1"], rsd, start=True, stop=True)
        brs_s = stats1_pool.tile([128, S], F32, tag="brs_s")
        nc.vector.tensor_copy(out=brs_s, in_=prs)
        for k in range(KC):
            t = stats_pool.tile([128, S], F32, tag="lnt")
            nc.vector.scalar_tensor_tensor(out=t, in0=bsum_s, scalar=-1.0 / D,
                                           in1=src[k], op0=OP.mult,
                                           op1=OP.add)
            nc.vector.tensor_tensor(out=t, in0=t, in1=brs_s, op=OP.mult)
            nc.scalar.activation(out=dst[k], in_=t, func=AF.Identity,
                                 bias=b_t[:, k:k + 1],
                                 scale=g_t[:, k:k + 1])


# =========================================================================
# Host side
# =========================================================================
_CACHE = {}


def _get_program():
    if "nc" not in _CACHE:
        _install_ntff_hook()
        _CACHE["nc"] = build_program(L)
    return _CACHE["nc"]


def make_in_maps(inputs, n_layers=L):
    tokens = np.asarray(inputs["tokens"])
    we = np.asarray(inputs["we"], dtype=np.float32)
    pos = we[V:V + S]                                  # [S, D]
    triu = np.triu(np.ones((128, 128), dtype=np.float32))

    def f32(name):
        return np.ascontiguousarray(np.asarray(inputs[name])[:n_layers],
                                    dtype=np.float32)

    shared = {k: f32(k) for k in ["wqkv", "bqkv", "wproj", "bproj", "g1", "b1",
                                  "wfc", "bfc", "wpr", "bpr", "g2", "b2"]}
    shared["triu"] = triu
    shared["ones_row"] = np.ones((1, 128), dtype=np.float32)
    onesred = np.zeros((128, 2), dtype=np.float32); onesred[:, 0] = 1.0
    shared["ones_red"] = onesred
    sel_den = np.zeros((128, 4, 4), dtype=np.float32)
    for j in range(4):
        sel_den[:, j, j] = 1.0
    shared["sel_den"] = sel_den
    sel_bc2 = np.zeros((4, 2, 128), dtype=np.float32)
    for q in range(2):
        sel_bc2[2 * q, q, 0:64] = 1.0
        sel_bc2[2 * q + 1, q, 64:128] = 1.0
    shared["sel_bc2"] = sel_bc2
    shared["ones2d"] = np.ones((128, 128), dtype=np.float32)
    shared["ident"] = np.eye(128, dtype=np.float32)
    in_maps = []
    for b in range(N_CORES):
        x0 = we[tokens[b]] + pos                       # [S, D]
        m = dict(shared)
        m["x0T"] = np.ascontiguousarray(x0.T, dtype=np.float32)
        in_maps.append(m)
    return in_maps


def run(inputs, trace=False):
    nc = _get_program()
    in_maps = make_in_maps(inputs)
    res = bass_utils.run_bass_kernel_spmd(nc, in_maps,
                                          core_ids=list(range(N_CORES)),
                                          trace=trace)
    outs = np.stack([res.results[b]["out"].T for b in range(N_CORES)])
    return outs.astype(np.float32), res


def kernel(**inputs):
    out, _ = run(inputs, trace=False)
    return out
